# revision 3
# baseline (speedup 1.0000x reference)
"""Trainium2 Bass kernel for nn_DGEBlock (dense transformer block with
MoE-gated linears), distributed over 8 NeuronCores.

Sharding: data-parallel over batch (2 groups of 4 cores) x sequence-parallel
over tokens within each batch (512 tokens per core). Weights are replicated
(host pre-tiled); activations live feature-major ("T-layout": [d, tok]) in
SBUF so projections are lhsT=W^T-tile @ rhs=activation with no activation
transposes. V is projected in token-major (N-)layout directly so attention's
PV matmuls need no transposes either.

FP8 (e4m3) with DoubleRow double-pumping on the TensorEngine for the
projection matmuls: q/k/v/o main+gate, mlp_in main+gate, mlp_out gate.
Weights are pre-scaled x16 on the host so their values sit in the fp8
normal range; descaling is deferred into existing epilogue scalars (ACT
`scale=` for sigmoid/gelu arguments, scalar_tensor_tensor for residual
adds), so the fp8 path adds no extra element-wise ops. mlp_out's main
matmul stays bf16 (its quantization error dominates the output). The
attention QK matmul runs in plain fp8 (K gathered as fp8); exp/PV stay
bf16 because logits can exceed the fp8 range after exp (no max trick).
"""

import sys

for _p in ("/opt/trn_rl_repo",):
    if _p not in sys.path:
        sys.path.append(_p)

import numpy as np
import ml_dtypes

# ---------------------------------------------------------------- constants
B = 2
T = 2048
D = 2048
H = 16
HD = 128
FF = 4 * D  # 8192
EPS = 1e-5

N_CORES = 8
GROUP = 4  # cores per batch group (sequence-parallel degree)
S = T // GROUP  # tokens per core = 512
P = 128
NT = D // P  # 16 feature tiles
NF = FF // P  # 64 hidden tiles
NKB = T // P  # 16 key blocks per batch
ISCALE = 1.0 / float(np.sqrt(HD))

WS = 16.0  # fp8 weight pre-scale
FP8_IN_MAIN = True  # mlp_in main matmul in fp8 (False -> bf16 fallback)

RG = [[0, 1, 2, 3], [4, 5, 6, 7]]

_BF = ml_dtypes.bfloat16
_F8 = ml_dtypes.float8_e4m3

_COMPILED = None


# ------------------------------------------------------------- host prep
def _tile4(W):
    """W [dout, din] -> [nj, 128, nt, 128] such that
    out[j, p, t, jc] == W[j*128+jc, t*128+p]  (= W^T tile (t, j))."""
    dout, din = W.shape
    nj, nt = dout // P, din // P
    return W.reshape(nj, P, nt, P).transpose(0, 3, 2, 1)


def _w_tiled_bf(W):
    return np.ascontiguousarray(_tile4(W).astype(_BF))


def _w_tiled_f8(W):
    return np.ascontiguousarray(
        np.clip(_tile4(W) * WS, -240, 240).astype(_F8)
    )


def _b_cols(b, scale=1.0):
    """b [dout] -> [128, nj] fp32: column j holds b[j*128:(j+1)*128]."""
    nj = b.shape[0] // P
    return np.ascontiguousarray((b * scale).reshape(nj, P).T.astype(np.float32))


# ------------------------------------------------------------- device build
def _build():
    from concourse import bacc, tile, mybir

    fp32 = mybir.dt.float32
    bf16 = mybir.dt.bfloat16
    f8 = mybir.dt.float8e4
    AF = mybir.ActivationFunctionType
    ALU = mybir.AluOpType
    DR = mybir.MatmulPerfMode.DoubleRow

    in_main_dt = f8 if FP8_IN_MAIN else bf16

    nc = bacc.Bacc("TRN2", target_bir_lowering=False, debug=False,
                   num_devices=N_CORES)

    # ---- I/O tensors
    xT_d = nc.dram_tensor("xT", [D, S], fp32, kind="ExternalInput")
    wd = {}
    for nm in ("Wq", "Wgq", "Wk", "Wgk", "Wo", "Wgo"):
        wd[nm] = nc.dram_tensor(nm, [NT, P, NT, P], f8, kind="ExternalInput")
    wd["Win"] = nc.dram_tensor("Win", [NF, P, NT, P], in_main_dt,
                               kind="ExternalInput")
    wd["Wgin"] = nc.dram_tensor("Wgin", [NF, P, NT, P], f8,
                                kind="ExternalInput")
    wd["Wout"] = nc.dram_tensor("Wout", [NT, P, NF, P], bf16,
                                kind="ExternalInput")
    wd["Wgout"] = nc.dram_tensor("Wgout", [NT, P, NF, P], f8,
                                 kind="ExternalInput")
    # V projection runs in N-layout: plain W^T [din, dout] + bias rows
    wd["WvT"] = nc.dram_tensor("WvT", [D, D], f8, kind="ExternalInput")
    wd["WgvT"] = nc.dram_tensor("WgvT", [D, D], f8, kind="ExternalInput")
    bvrow_d = nc.dram_tensor("bvrow", [1, D], bf16, kind="ExternalInput")
    bgvrow_d = nc.dram_tensor("bgvrow", [1, D], bf16, kind="ExternalInput")
    bd = {}
    for nm in ("bq", "bgq", "bk", "bgk", "bo", "bgo",
               "bout", "bgout", "g1", "bt1", "g2", "bt2"):
        bd[nm] = nc.dram_tensor(nm, [P, NT], fp32, kind="ExternalInput")
    for nm in ("bin", "bgin"):
        bd[nm] = nc.dram_tensor(nm, [P, NF], fp32, kind="ExternalInput")
    out_d = nc.dram_tensor("outT", [D, S], fp32, kind="ExternalOutput")

    with tile.TileContext(nc) as tc:
        with (
            tc.tile_pool(name="const", bufs=1) as constp,
            tc.tile_pool(name="bias", bufs=1) as biasp,
            tc.tile_pool(name="rows", bufs=1) as rows,
            tc.tile_pool(name="dram", bufs=1, space="DRAM") as dramp,
        ):
            ones_col = constp.tile([P, 1], bf16)
            nc.vector.memset(ones_col[:], 1.0)
            ones_row = constp.tile([1, P], bf16)
            nc.vector.memset(ones_row[:], 1.0)
            eps_t = constp.tile([1, 1], fp32)
            nc.vector.memset(eps_t[:], EPS)
            invo = constp.tile([P, 1], fp32)
            nc.vector.memset(invo[:], 1.0 / (WS * WS))
            bvrow = constp.tile([1, D], bf16)
            nc.sync.dma_start(bvrow[:], bvrow_d.ap())
            bgvrow = constp.tile([1, D], bf16)
            nc.sync.dma_start(bgvrow[:], bgvrow_d.ap())

            bias = {}
            for nm in bd:
                ncols = NF if nm in ("bin", "bgin") else NT
                btile = biasp.tile([P, ncols], fp32, name=f"bias_{nm}")
                nc.sync.dma_start(btile[:], bd[nm].ap())
                bias[nm] = btile

            # ---------- helpers ----------
            def ln_T(src, gname, bname, hpool, tmpool, psln, name,
                     out_dt=f8):
                """LayerNorm over the feature dim of a T-layout activation.

                src: SBUF tile [128, NT, S] fp32 -> returns out_dt [128, NT, S].
                Stats via ones-matmuls (contract over partitions); per-token
                scale/shift rows are broadcast to [128, S] via rank-1 matmuls.
                """
                S1 = psln.tile([1, S], fp32, name=f"{name}_S1", tag="ln_S1")
                S2 = psln.tile([1, S], fp32, name=f"{name}_S2", tag="ln_S2")
                for t in range(NT):
                    xbf = tmpool.tile([P, S], bf16, name=f"{name}_xbf_{t}",
                                      tag="ln_xbf", bufs=3)
                    nc.vector.tensor_copy(xbf[:], src[:, t, :])
                    sq = tmpool.tile([P, S], bf16, name=f"{name}_sq_{t}",
                                     tag="ln_sq", bufs=3)
                    nc.scalar.activation(sq[:], src[:, t, :], AF.Square)
                    nc.tensor.matmul(S1[:], ones_col[:], xbf[:],
                                     start=(t == 0), stop=(t == NT - 1))
                    nc.tensor.matmul(S2[:], ones_col[:], sq[:],
                                     start=(t == 0), stop=(t == NT - 1))

                def row(nm, dt=fp32):
                    return rows.tile([1, S], dt, name=f"{name}_{nm}",
                                     tag=f"ln_{nm}")

                mean = row("mean")
                nc.vector.tensor_scalar_mul(mean[:], S1[:], 1.0 / D)
                m2 = row("m2")
                nc.vector.tensor_scalar_mul(m2[:], S2[:], 1.0 / D)
                msq = row("msq")
                nc.vector.tensor_tensor(msq[:], mean[:], mean[:],
                                        op=ALU.mult)
                var = row("var")
                nc.vector.tensor_tensor(var[:], m2[:], msq[:],
                                        op=ALU.subtract)
                std = row("std")
                nc.scalar.activation(std[:], var[:], AF.Sqrt,
                                     bias=eps_t[:])
                rstd = row("rstd")
                nc.vector.reciprocal(rstd[:], std[:])
                rstd_bf = row("rstdbf", bf16)
                nc.vector.tensor_copy(rstd_bf[:], rstd[:])
                mr_bf = row("mrbf", bf16)
                nc.vector.tensor_tensor(mr_bf[:], mean[:], rstd[:],
                                        op=ALU.mult)
                Ab_p = psln.tile([P, S], fp32, name=f"{name}_Abp",
                                 tag="ln_Abp")
                nc.tensor.matmul(Ab_p[:], ones_row[:], rstd_bf[:])
                Bb_p = psln.tile([P, S], fp32, name=f"{name}_Bbp",
                                 tag="ln_Bbp")
                nc.tensor.matmul(Bb_p[:], ones_row[:], mr_bf[:])
                Ab = tmpool.tile([P, S], fp32, name=f"{name}_Ab")
                nc.vector.tensor_copy(Ab[:], Ab_p[:])
                Bb = tmpool.tile([P, S], fp32, name=f"{name}_Bb")
                nc.vector.tensor_copy(Bb[:], Bb_p[:])
                h = hpool.tile([P, NT, S], out_dt, name=f"{name}_h")
                for t in range(NT):
                    tmp = tmpool.tile([P, S], fp32, name=f"{name}_t0_{t}",
                                      tag="ln_t0", bufs=3)
                    nc.vector.tensor_tensor(tmp[:], src[:, t, :], Ab[:],
                                            op=ALU.mult)
                    tmp2 = tmpool.tile([P, S], fp32, name=f"{name}_t1_{t}",
                                       tag="ln_t1", bufs=3)
                    nc.vector.tensor_tensor(tmp2[:], tmp[:], Bb[:],
                                            op=ALU.subtract)
                    nc.scalar.activation(h[:, t, :], tmp2[:], AF.Identity,
                                         bias=bias[bname][:, t:t + 1],
                                         scale=bias[gname][:, t:t + 1])
                return h

            def accum(psum, src, wname, j, nt, tchunk, wpool, wbufs, fp8,
                      tag):
                """psum += sum_t W^T(t,j).T @ src[:,t,:]; fp8 uses DoubleRow
                over k-tile pairs."""
                nchunk = nt // tchunk
                wdt = f8 if fp8 else bf16
                for ci in range(nchunk):
                    wt = wpool.tile([P, tchunk, P], wdt, tag=tag,
                                    name=f"w_{wname}_{j}_{ci}", bufs=wbufs)
                    nc.sync.dma_start(
                        wt[:],
                        wd[wname].ap()[j, :,
                                       ci * tchunk:(ci + 1) * tchunk, :])
                    if fp8:
                        for ti in range(0, tchunk, 2):
                            t = ci * tchunk + ti
                            nc.tensor.matmul(psum[:], wt[:, ti:ti + 2, :],
                                             src[:, t:t + 2, :],
                                             start=(t == 0),
                                             stop=(t == nt - 2),
                                             perf_mode=DR)
                    else:
                        for ti in range(tchunk):
                            t = ci * tchunk + ti
                            nc.tensor.matmul(psum[:], wt[:, ti, :],
                                             src[:, t, :],
                                             start=(t == 0),
                                             stop=(t == nt - 1))

            def proj_gated(src_m, src_g, nt, nj, wname, wgname, bgname,
                           wpool, pspool, epilogue, gsc, m_fp8=True,
                           tchunk=None, wbufs=3):
                """Gated projection in T-layout. Gate path is always fp8;
                main path fp8 iff m_fp8. sig = sigmoid(gate*gsc + bg)."""
                if tchunk is None:
                    tchunk = nt
                for j in range(nj):
                    main = pspool.tile([P, S], fp32, name=f"{wname}_m{j}",
                                       tag="pj_main", bufs=2)
                    gate = pspool.tile([P, S], fp32, name=f"{wname}_g{j}",
                                       tag="pj_gate", bufs=2)
                    accum(main, src_m, wname, j, nt, tchunk, wpool, wbufs,
                          m_fp8, "wmain")
                    accum(gate, src_g, wgname, j, nt, tchunk, wpool, wbufs,
                          True, "wgate")
                    sig = wpool.tile([P, S], bf16, tag="sig",
                                     name=f"sig_{wname}_{j}", bufs=3)
                    nc.scalar.activation(sig[:], gate[:], AF.Sigmoid,
                                         bias=bias[bgname][:, j:j + 1],
                                         scale=gsc)
                    epilogue(j, main, sig)

            # x2 outlives phases A-C (used by LN2 + MLP residual)
            with tc.tile_pool(name="x2p", bufs=1) as x2p:
              with tc.tile_pool(name="xt", bufs=1) as xtp:
                xt = xtp.tile([P, NT, S], fp32)
                xT_v = xT_d.ap().rearrange("(t p) s -> t p s", p=P)
                for t in range(NT):
                    nc.sync.dma_start(xt[:, t, :], xT_v[t])

                vN_bounce = dramp.tile([S, D], bf16)
                k_bounce = dramp.tile([D, S], f8)
                vgN = dramp.tile([GROUP * S, D], bf16)
                kg = dramp.tile([GROUP * D, S], f8)

                with tc.tile_pool(name="yp", bufs=1) as ypool:
                  with tc.tile_pool(name="qp", bufs=1) as qpool:
                    q = qpool.tile([P, NT, S], f8)

                    with tc.tile_pool(name="hq", bufs=1) as hqp:
                        with (
                            tc.tile_pool(name="ln1tmp", bufs=1) as ln1tmp,
                            tc.tile_pool(name="ln1ps", bufs=1,
                                         space="PSUM") as ln1ps,
                        ):
                            h1 = ln_T(xt, "g1", "bt1", hqp, ln1tmp, ln1ps,
                                      "ln1")

                        # ---- V projection, N-layout ----
                        with (
                            tc.tile_pool(name="wv", bufs=1) as wvp,
                            tc.tile_pool(name="vps", bufs=1,
                                         space="PSUM") as vps,
                        ):
                            TC = NT // 2
                            for n in range(4):
                                vmain = [vps.tile([P, S], fp32,
                                                  tag="v_main", bufs=4,
                                                  name=f"vm_{n}_{m}")
                                         for m in range(4)]
                                vgate = [vps.tile([P, S], fp32,
                                                  tag="v_gate", bufs=4,
                                                  name=f"vg_{n}_{m}")
                                         for m in range(4)]
                                for ci in range(2):
                                    wvt = wvp.tile([P, TC, 4 * P], f8,
                                                   tag="wv", bufs=2,
                                                   name=f"wv_{n}_{ci}")
                                    wgvt = wvp.tile([P, TC, 4 * P], f8,
                                                    tag="wgv", bufs=2,
                                                    name=f"wgv_{n}_{ci}")
                                    for ti in range(TC):
                                        t = ci * TC + ti
                                        nc.sync.dma_start(
                                            wvt[:, ti, :],
                                            wd["WvT"].ap()[t * P:(t + 1) * P,
                                                           n * S:(n + 1) * S])
                                        nc.sync.dma_start(
                                            wgvt[:, ti, :],
                                            wd["WgvT"].ap()[
                                                t * P:(t + 1) * P,
                                                n * S:(n + 1) * S])
                                    for m in range(4):
                                        for ti in range(0, TC, 2):
                                            t = ci * TC + ti
                                            nc.tensor.matmul(
                                                vmain[m][:],
                                                h1[:, t:t + 2,
                                                   m * P:(m + 1) * P],
                                                wvt[:, ti:ti + 2, :],
                                                start=(t == 0), stop=False,
                                                perf_mode=DR)
                                        for ti in range(0, TC, 2):
                                            t = ci * TC + ti
                                            nc.tensor.matmul(
                                                vgate[m][:],
                                                h1[:, t:t + 2,
                                                   m * P:(m + 1) * P],
                                                wgvt[:, ti:ti + 2, :],
                                                start=(t == 0), stop=False,
                                                perf_mode=DR)
                                for m in range(4):
                                    nc.tensor.matmul(
                                        vmain[m][:], ones_row[:],
                                        bvrow[:, n * S:(n + 1) * S],
                                        start=False, stop=True)
                                    nc.tensor.matmul(
                                        vgate[m][:], ones_row[:],
                                        bgvrow[:, n * S:(n + 1) * S],
                                        start=False, stop=True)
                                    vsig = wvp.tile([P, S], bf16,
                                                    tag="vsig", bufs=3,
                                                    name=f"vsig_{n}_{m}")
                                    nc.scalar.activation(vsig[:],
                                                         vgate[m][:],
                                                         AF.Sigmoid,
                                                         scale=1.0 / WS)
                                    vout = wvp.tile([P, S], bf16,
                                                    tag="vout", bufs=3,
                                                    name=f"vout_{n}_{m}")
                                    nc.vector.tensor_tensor(
                                        vout[:], vmain[m][:], vsig[:],
                                        op=ALU.mult)
                                    nc.scalar.dma_start(
                                        vN_bounce[m * P:(m + 1) * P,
                                                  n * S:(n + 1) * S],
                                        vout[:])

                        nc.gpsimd.collective_compute(
                            "AllGather", ALU.bypass, ins=[vN_bounce[:]],
                            outs=[vgN[:]], replica_groups=RG)

                        # ---- K projection (T-layout) + AllGather ----
                        with (
                            tc.tile_pool(name="wproj", bufs=1) as wpool,
                            tc.tile_pool(name="pjps", bufs=1,
                                         space="PSUM") as pjps,
                        ):
                            def k_epi(j, main, sig):
                                kv = wpool.tile([P, S], f8, tag="kv_out",
                                                name=f"kv_k_{j}", bufs=3)
                                nc.vector.scalar_tensor_tensor(
                                    kv[:], main[:], bias["bk"][:, j:j + 1],
                                    sig[:], op0=ALU.add, op1=ALU.mult)
                                nc.scalar.dma_start(
                                    k_bounce[j * P:(j + 1) * P, :], kv[:])

                            proj_gated(h1, h1, NT, NT, "Wk", "Wgk", "bgk",
                                       wpool, pjps, k_epi, 1.0 / WS)

                            nc.gpsimd.collective_compute(
                                "AllGather", ALU.bypass, ins=[k_bounce[:]],
                                outs=[kg[:]], replica_groups=RG)

                            def q_epi(j, main, sig):
                                nc.vector.scalar_tensor_tensor(
                                    q[:, j, :], main[:],
                                    bias["bq"][:, j:j + 1],
                                    sig[:], op0=ALU.add, op1=ALU.mult)

                            proj_gated(h1, h1, NT, NT, "Wq", "Wgq", "bgq",
                                       wpool, pjps, q_epi, 1.0 / WS)

                    # ---- phase B: attention ----
                    with (
                        tc.tile_pool(name="vres", bufs=1) as vresp,
                        tc.tile_pool(name="kstream", bufs=2) as kpool,
                        tc.tile_pool(name="apool", bufs=4) as apool,
                        tc.tile_pool(name="atps", bufs=1,
                                     space="PSUM") as atps,
                    ):
                        y = ypool.tile([P, NT, S], f8)
                        # V resident [k-part, kb, d]; plain loads from the
                        # gathered N-layout V, issued on the GpSimd queue.
                        Vt = vresp.tile([P, NKB, D], bf16)
                        for kb in range(NKB):
                            nc.gpsimd.dma_start(
                                Vt[:, kb, :],
                                vgN[kb * P:(kb + 1) * P, :])

                        head_state = {}

                        def finalize_head(h, Zp_h, Yp_h):
                            urow = rows.tile([1, S], fp32, name=f"u_{h}",
                                             tag="urow", bufs=2)
                            nc.vector.reciprocal(urow[:], Zp_h[:])
                            ubf = rows.tile([1, S], bf16, name=f"ubf_{h}",
                                            tag="ubf", bufs=2)
                            nc.vector.tensor_copy(ubf[:], urow[:])
                            Up = atps.tile([P, S], fp32, name=f"Up_{h}",
                                           tag="logits", bufs=4)
                            nc.tensor.matmul(Up[:], ones_row[:], ubf[:])
                            Us = apool.tile([P, S], bf16, tag="Us",
                                            name=f"Us_{h}")
                            nc.vector.tensor_copy(Us[:], Up[:])
                            nc.vector.tensor_tensor(y[:, h, :], Yp_h[:],
                                                    Us[:], op=ALU.mult)

                        for hh in range(H):
                            Kh = kpool.tile([P, NKB * P], f8, tag="Kh",
                                            name=f"Kh_{hh}")
                            for s_ in range(GROUP):
                                nc.gpsimd.dma_start(
                                    Kh[:, s_ * S:(s_ + 1) * S],
                                    kg[s_ * D + hh * P:
                                       s_ * D + (hh + 1) * P, :])
                            Zp = atps.tile([1, S], fp32, name=f"Z_{hh}",
                                           tag="Zp", bufs=2)
                            Yp = atps.tile([P, S], fp32, name=f"Y_{hh}",
                                           tag="Yp", bufs=2)
                            ats = {}

                            def do_L(kb, hh=hh, Kh=Kh, ats=ats):
                                Lp = atps.tile([P, S], fp32,
                                               name=f"L_{hh}_{kb}",
                                               tag="logits", bufs=4)
                                nc.tensor.matmul(
                                    Lp[:], Kh[:, kb * P:(kb + 1) * P],
                                    q[:, hh, :])
                                At = apool.tile([P, S], bf16, tag="At",
                                                name=f"At_{hh}_{kb}",
                                                bufs=6)
                                nc.scalar.activation(
                                    At[:], Lp[:], AF.Exp,
                                    scale=ISCALE / (WS * WS))
                                ats[kb] = At

                            do_L(0)
                            do_L(1)
                            for kb in range(NKB):
                                if kb + 2 < NKB:
                                    do_L(kb + 2)
                                nc.tensor.matmul(Zp[:], ones_col[:],
                                                 ats[kb][:],
                                                 start=(kb == 0),
                                                 stop=(kb == NKB - 1))
                                nc.tensor.matmul(
                                    Yp[:],
                                    Vt[:, kb, hh * P:(hh + 1) * P],
                                    ats[kb][:],
                                    start=(kb == 0),
                                    stop=(kb == NKB - 1))
                                if kb == 3 and hh > 0:
                                    finalize_head(hh - 1,
                                                  *head_state[hh - 1])
                            head_state[hh] = (Zp, Yp)
                        finalize_head(H - 1, *head_state[H - 1])

                  # ---- phase C: o-proj + residual ----
                  x2 = x2p.tile([P, NT, S], fp32, name="x2")
                  with (
                      tc.tile_pool(name="wproj2", bufs=1) as wpool2,
                      tc.tile_pool(name="pj2ps", bufs=1,
                                   space="PSUM") as pj2ps,
                  ):
                      def o_epi(j, main, sig):
                          tmp = wpool2.tile([P, S], fp32, tag="o_tmp",
                                            name=f"o_tmp_{j}", bufs=3)
                          nc.vector.scalar_tensor_tensor(
                              tmp[:], main[:], bias["bo"][:, j:j + 1],
                              sig[:], op0=ALU.add, op1=ALU.mult)
                          nc.vector.scalar_tensor_tensor(
                              x2[:, j, :], tmp[:], invo[:],
                              xt[:, j, :], op0=ALU.mult, op1=ALU.add)

                      proj_gated(y, y, NT, NT, "Wo", "Wgo", "bgo",
                                 wpool2, pj2ps, o_epi, 1.0 / (WS * WS))

              # ---- phase D: LN2 + MLP ----
              with (
                  tc.tile_pool(name="midp", bufs=1) as midp,
                  tc.tile_pool(name="mid8p", bufs=1) as mid8p,
              ):
                  mid = midp.tile([P, NF, S], bf16)
                  mid8 = mid8p.tile([P, NF, S], f8)
                  with tc.tile_pool(name="h2p", bufs=1) as h2p:
                      with (
                          tc.tile_pool(name="ln2tmp", bufs=1) as ln2tmp,
                          tc.tile_pool(name="ln2ps", bufs=1,
                                       space="PSUM") as ln2ps,
                      ):
                          h2 = ln_T(x2, "g2", "bt2", h2p, ln2tmp, ln2ps,
                                    "ln2", out_dt=in_main_dt)
                          if FP8_IN_MAIN:
                              h2m = h2g = h2
                          else:
                              h2g8 = h2p.tile([P, NT, S], f8, name="h2g8")
                              for t in range(NT):
                                  nc.vector.tensor_copy(h2g8[:, t, :],
                                                        h2[:, t, :])
                              h2m, h2g = h2, h2g8

                      with (
                          tc.tile_pool(name="wmlp1", bufs=1) as wm1,
                          tc.tile_pool(name="m1ps", bufs=1,
                                       space="PSUM") as m1ps,
                      ):
                          def mid_epi(j, main, sig):
                              tmp = wm1.tile([P, S], fp32, tag="mid_tmp",
                                             name=f"mid_tmp_{j}", bufs=3)
                              nc.vector.scalar_tensor_tensor(
                                  tmp[:], main[:],
                                  bias["bin"][:, j:j + 1], sig[:],
                                  op0=ALU.add, op1=ALU.mult)
                              nc.scalar.activation(
                                  mid[:, j, :], tmp[:], AF.Gelu,
                                  scale=(1.0 / WS if FP8_IN_MAIN else 1.0))
                              nc.vector.tensor_copy(mid8[:, j, :],
                                                    mid[:, j, :])

                          proj_gated(h2m, h2g, NT, NF, "Win", "Wgin",
                                     "bgin", wm1, m1ps, mid_epi, 1.0 / WS,
                                     m_fp8=FP8_IN_MAIN)

                  with (
                      tc.tile_pool(name="wmlp2", bufs=1) as wm2,
                      tc.tile_pool(name="m2ps", bufs=1,
                                   space="PSUM") as m2ps,
                  ):
                      def out_epi(j, main, sig):
                          tmp = wm2.tile([P, S], fp32, tag="out_tmp",
                                         name=f"out_tmp_{j}", bufs=3)
                          nc.vector.scalar_tensor_tensor(
                              tmp[:], main[:], bias["bout"][:, j:j + 1],
                              sig[:], op0=ALU.add, op1=ALU.mult)
                          outf = wm2.tile([P, S], fp32, tag="out_f",
                                          name=f"out_f_{j}", bufs=3)
                          nc.vector.tensor_tensor(outf[:], tmp[:],
                                                  x2[:, j, :], op=ALU.add)
                          nc.sync.dma_start(
                              out_d.ap()[j * P:(j + 1) * P, :], outf[:])

                      proj_gated(mid, mid8, NF, NT, "Wout", "Wgout",
                                 "bgout", wm2, m2ps, out_epi, 1.0 / WS,
                                 m_fp8=False, tchunk=32, wbufs=2)

    nc.compile()
    return nc


def _prep_shared_inputs(inputs):
    m = {}
    for nm, w in (("Wq", "W_q"), ("Wgq", "Wg_q"), ("Wk", "W_k"),
                  ("Wgk", "Wg_k"), ("Wo", "W_o"), ("Wgo", "Wg_o"),
                  ("Wgin", "Wg_in"), ("Wgout", "Wg_out")):
        m[nm] = _w_tiled_f8(np.asarray(inputs[w]))
    m["Win"] = (_w_tiled_f8 if FP8_IN_MAIN else _w_tiled_bf)(
        np.asarray(inputs["W_in"]))
    m["Wout"] = _w_tiled_bf(np.asarray(inputs["W_out"]))
    m["WvT"] = np.ascontiguousarray(
        np.clip(np.asarray(inputs["W_v"]).T * WS, -240, 240).astype(_F8))
    m["WgvT"] = np.ascontiguousarray(
        np.clip(np.asarray(inputs["Wg_v"]).T * WS, -240, 240).astype(_F8))
    m["bvrow"] = (np.asarray(inputs["b_v"]) * WS).astype(_BF).reshape(1, D)
    m["bgvrow"] = (np.asarray(inputs["bg_v"]) * WS).astype(_BF).reshape(1, D)
    main_b_scale = {"bq": WS, "bk": WS, "bo": WS * WS,
                    "bin": (WS if FP8_IN_MAIN else 1.0), "bout": 1.0}
    for nm, bn in (("bq", "b_q"), ("bgq", "bg_q"), ("bk", "b_k"),
                   ("bgk", "bg_k"), ("bo", "b_o"), ("bgo", "bg_o"),
                   ("bin", "b_in"), ("bgin", "bg_in"), ("bout", "b_out"),
                   ("bgout", "bg_out"), ("g1", "ln1_g"), ("bt1", "ln1_b"),
                   ("g2", "ln2_g"), ("bt2", "ln2_b")):
        m[nm] = _b_cols(np.asarray(inputs[bn]), main_b_scale.get(nm, 1.0))
    return m


def _install_trace_shim():
    """Provide antenv.axon_hooks (NTFF profiling) if the image lacks it."""
    import contextlib
    import ctypes
    import types

    try:
        import antenv.axon_hooks  # noqa: F401
        return
    except ImportError:
        pass
    try:
        import antenv
    except ImportError:
        return
    so_path = "/opt/axon/libaxon_pjrt.so"
    try:
        lib = ctypes.CDLL(so_path)
    except OSError:
        return
    if not hasattr(lib, "axon_start_nrt_profile"):
        return
    lib.axon_start_nrt_profile.argtypes = [ctypes.POINTER(ctypes.c_int64),
                                           ctypes.c_size_t]
    lib.axon_start_nrt_profile.restype = ctypes.c_int64
    lib.axon_stop_nrt_profile.argtypes = [ctypes.c_char_p]
    lib.axon_stop_nrt_profile.restype = ctypes.c_int64

    @contextlib.contextmanager
    def hook(output_dir, device_ids):
        import jax

        jax.devices()
        if device_ids:
            ids = (ctypes.c_int64 * len(device_ids))(*device_ids)
            rc = lib.axon_start_nrt_profile(ids, len(device_ids))
        else:
            rc = lib.axon_start_nrt_profile(None, 0)
        if rc != 0:
            raise RuntimeError(f"axon_start_nrt_profile rc={rc}")
        try:
            yield
        finally:
            n = lib.axon_stop_nrt_profile(str(output_dir).encode())
            print(f"profile: {n} ntff file(s) in {output_dir}",
                  file=sys.stderr)

    mod = types.ModuleType("antenv.axon_hooks")
    mod.get_axon_ntff_profile_hook = lambda: hook
    mod.set_axon_ntff_profile_hook = lambda h: None
    sys.modules["antenv.axon_hooks"] = mod
    antenv.axon_hooks = mod


LAST_RESULTS = None


def kernel(_trace=False, **inputs):
    global _COMPILED, LAST_RESULTS
    from concourse import bass_utils

    if _trace:
        _install_trace_shim()

    if _COMPILED is None:
        _COMPILED = _build()
    nc = _COMPILED

    shared = _prep_shared_inputs(inputs)
    x = np.asarray(inputs["x"], dtype=np.float32)  # [B, T, D]
    in_maps = []
    for c in range(N_CORES):
        g, s = divmod(c, GROUP)
        xT_c = np.ascontiguousarray(x[g, s * S:(s + 1) * S, :].T)
        m = dict(shared)
        m["xT"] = xT_c
        in_maps.append(m)

    LAST_RESULTS = bass_utils.run_bass_kernel_spmd(
        nc, in_maps, core_ids=list(range(N_CORES)), trace=_trace)

    out = np.empty((B, T, D), dtype=np.float32)
    for c in range(N_CORES):
        g, s = divmod(c, GROUP)
        out[g, s * S:(s + 1) * S, :] = LAST_RESULTS.results[c]["outT"].T
    return out


# revision 10
# speedup vs baseline: 1.0057x; 1.0057x over previous
"""Trainium2 Bass kernel for nn_DGEBlock (dense transformer block with
MoE-gated linears), distributed over 8 NeuronCores.

Sharding: data-parallel over batch (2 groups of 4 cores) x sequence-parallel
over tokens within each batch (512 tokens per core). Weights are replicated
(host pre-tiled); activations live feature-major ("T-layout": [d, tok]) in
SBUF so projections are lhsT=W^T-tile @ rhs=activation with no activation
transposes. V is projected in token-major (N-)layout directly so attention's
PV matmuls need no transposes either.

FP8 (e4m3) with DoubleRow double-pumping on the TensorEngine for the
projection matmuls: q/k/v/o main+gate, mlp_in main+gate, mlp_out gate.
Weights are pre-scaled x16 on the host so their values sit in the fp8
normal range; descaling is deferred into existing epilogue scalars. The
LN affine (gain/shift) is folded into the downstream weights host-side,
so LN on-device is only stats + a 2-op DVE normalize per tile. mlp_out's
main matmul stays bf16 (its quantization error dominates the output).
Attention: QK in plain fp8, exp/PV in bf16 (exp range exceeds fp8), the
softmax denominator accumulated on the Vector engine (frees the PE),
collectives gathered in fp8 with K-proj scheduled first so both
AllGathers hide under the V/Q projections.
"""

import sys

for _p in ("/opt/trn_rl_repo",):
    if _p not in sys.path:
        sys.path.append(_p)

import numpy as np
import ml_dtypes

# ---------------------------------------------------------------- constants
B = 2
T = 2048
D = 2048
H = 16
HD = 128
FF = 4 * D  # 8192
EPS = 1e-5

N_CORES = 8
GROUP = 4  # cores per batch group (sequence-parallel degree)
S = T // GROUP  # tokens per core = 512
P = 128
NT = D // P  # 16 feature tiles
NF = FF // P  # 64 hidden tiles
NKB = T // P  # 16 key blocks per batch
ISCALE = 1.0 / float(np.sqrt(HD))

WS = 16.0  # fp8 weight pre-scale
FP8_IN_MAIN = True  # mlp_in main matmul in fp8 (False -> bf16 fallback)

RG = [[0, 1, 2, 3], [4, 5, 6, 7]]

_BF = ml_dtypes.bfloat16
_F8 = ml_dtypes.float8_e4m3

_COMPILED = None


# ------------------------------------------------------------- host prep
def _tile4(W):
    """W [dout, din] -> [nj, 128, nt, 128] such that
    out[j, p, t, jc] == W[j*128+jc, t*128+p]  (= W^T tile (t, j))."""
    dout, din = W.shape
    nj, nt = dout // P, din // P
    return W.reshape(nj, P, nt, P).transpose(0, 3, 2, 1)


def _w_tiled_bf(W):
    return np.ascontiguousarray(_tile4(W).astype(_BF))


def _w_tiled_f8(W):
    return np.ascontiguousarray(
        np.clip(_tile4(W) * WS, -240, 240).astype(_F8)
    )


def _b_cols(b, scale=1.0):
    """b [dout] -> [128, nj] fp32: column j holds b[j*128:(j+1)*128]."""
    nj = b.shape[0] // P
    return np.ascontiguousarray((b * scale).reshape(nj, P).T.astype(np.float32))


# ------------------------------------------------------------- device build
def _build():
    from concourse import bacc, tile, mybir

    fp32 = mybir.dt.float32
    bf16 = mybir.dt.bfloat16
    f8 = mybir.dt.float8e4
    AF = mybir.ActivationFunctionType
    ALU = mybir.AluOpType
    DR = mybir.MatmulPerfMode.DoubleRow

    in_main_dt = f8 if FP8_IN_MAIN else bf16

    nc = bacc.Bacc("TRN2", target_bir_lowering=False, debug=False,
                   num_devices=N_CORES)

    # ---- I/O tensors
    xT_d = nc.dram_tensor("xT", [D, S], fp32, kind="ExternalInput")
    wd = {}
    for nm in ("Wq", "Wgq", "Wk", "Wgk", "Wo", "Wgo"):
        wd[nm] = nc.dram_tensor(nm, [NT, P, NT, P], f8, kind="ExternalInput")
    wd["Win"] = nc.dram_tensor("Win", [NF, P, NT, P], in_main_dt,
                               kind="ExternalInput")
    wd["Wgin"] = nc.dram_tensor("Wgin", [NF, P, NT, P], f8,
                                kind="ExternalInput")
    wd["Wout"] = nc.dram_tensor("Wout", [NT, P, NF, P], bf16,
                                kind="ExternalInput")
    wd["Wgout"] = nc.dram_tensor("Wgout", [NT, P, NF, P], f8,
                                 kind="ExternalInput")
    # V projection runs in N-layout: plain W^T [din, dout] + bias rows
    wd["WvT"] = nc.dram_tensor("WvT", [D, D], f8, kind="ExternalInput")
    wd["WgvT"] = nc.dram_tensor("WgvT", [D, D], f8, kind="ExternalInput")
    bvrow_d = nc.dram_tensor("bvrow", [1, D], bf16, kind="ExternalInput")
    bgvrow_d = nc.dram_tensor("bgvrow", [1, D], bf16, kind="ExternalInput")
    bd = {}
    for nm in ("bq", "bgq", "bk", "bgk", "bo", "bgo", "bout", "bgout"):
        bd[nm] = nc.dram_tensor(nm, [P, NT], fp32, kind="ExternalInput")
    for nm in ("bin", "bgin"):
        bd[nm] = nc.dram_tensor(nm, [P, NF], fp32, kind="ExternalInput")
    out_d = nc.dram_tensor("outT", [D, S], fp32, kind="ExternalOutput")

    with tile.TileContext(nc) as tc:
        with (
            tc.tile_pool(name="const", bufs=1) as constp,
            tc.tile_pool(name="bias", bufs=1) as biasp,
            tc.tile_pool(name="rows", bufs=1) as rows,
            tc.tile_pool(name="dram", bufs=1, space="DRAM") as dramp,
        ):
            ones_col = constp.tile([P, 1], bf16)
            nc.vector.memset(ones_col[:], 1.0)
            ones_col_f = constp.tile([P, 1], fp32)
            nc.vector.memset(ones_col_f[:], 1.0)
            ones_row = constp.tile([1, P], bf16)
            nc.vector.memset(ones_row[:], 1.0)
            eps_t = constp.tile([1, 1], fp32)
            nc.vector.memset(eps_t[:], EPS)
            invo = constp.tile([P, 1], fp32)
            nc.vector.memset(invo[:], 1.0 / (WS * WS))
            bvrow = constp.tile([1, D], bf16)
            nc.sync.dma_start(bvrow[:], bvrow_d.ap())
            bgvrow = constp.tile([1, D], bf16)
            nc.sync.dma_start(bgvrow[:], bgvrow_d.ap())

            bias = {}
            for nm in bd:
                ncols = NF if nm in ("bin", "bgin") else NT
                btile = biasp.tile([P, ncols], fp32, name=f"bias_{nm}")
                nc.sync.dma_start(btile[:], bd[nm].ap())
                bias[nm] = btile

            # ---------- helpers ----------
            def ln_T(src, hpool, tmpool, psln, name, out_dt=f8):
                """LayerNorm (stats+normalize only; gain/shift folded into
                the consumer weights host-side).

                src: SBUF tile [128, NT, S] fp32 -> out_dt [128, NT, S].
                Stats via ones-matmuls (contract over partitions); per-token
                rstd / mean*rstd rows broadcast via rank-1 matmuls; then a
                2-op DVE normalize per tile.
                """
                S1 = psln.tile([1, S], fp32, name=f"{name}_S1", tag="ln_S1")
                S2 = psln.tile([1, S], fp32, name=f"{name}_S2", tag="ln_S2")
                for t in range(NT):
                    xbf = tmpool.tile([P, S], bf16, name=f"{name}_xbf_{t}",
                                      tag="ln_xbf", bufs=3)
                    nc.vector.tensor_copy(xbf[:], src[:, t, :])
                    sq = tmpool.tile([P, S], bf16, name=f"{name}_sq_{t}",
                                     tag="ln_sq", bufs=3)
                    nc.scalar.activation(sq[:], src[:, t, :], AF.Square)
                    nc.tensor.matmul(S1[:], ones_col[:], xbf[:],
                                     start=(t == 0), stop=(t == NT - 1))
                    nc.tensor.matmul(S2[:], ones_col[:], sq[:],
                                     start=(t == 0), stop=(t == NT - 1))

                def row(nm, dt=fp32):
                    return rows.tile([1, S], dt, name=f"{name}_{nm}",
                                     tag=f"ln_{nm}")

                mean = row("mean")
                nc.vector.tensor_scalar_mul(mean[:], S1[:], 1.0 / D)
                m2 = row("m2")
                nc.vector.tensor_scalar_mul(m2[:], S2[:], 1.0 / D)
                msq = row("msq")
                nc.vector.tensor_tensor(msq[:], mean[:], mean[:],
                                        op=ALU.mult)
                var = row("var")
                nc.vector.tensor_tensor(var[:], m2[:], msq[:],
                                        op=ALU.subtract)
                std = row("std")
                nc.scalar.activation(std[:], var[:], AF.Sqrt,
                                     bias=eps_t[:])
                rstd = row("rstd")
                nc.vector.reciprocal_approx_fast(rstd[:], std[:])
                rstd_bf = row("rstdbf", bf16)
                nc.vector.tensor_copy(rstd_bf[:], rstd[:])
                mr_bf = row("mrbf", bf16)
                nc.vector.tensor_tensor(mr_bf[:], mean[:], rstd[:],
                                        op=ALU.mult)
                Ab_p = psln.tile([P, S], fp32, name=f"{name}_Abp",
                                 tag="ln_Abp")
                nc.tensor.matmul(Ab_p[:], ones_row[:], rstd_bf[:])
                Bb_p = psln.tile([P, S], fp32, name=f"{name}_Bbp",
                                 tag="ln_Bbp")
                nc.tensor.matmul(Bb_p[:], ones_row[:], mr_bf[:])
                Ab = tmpool.tile([P, S], fp32, name=f"{name}_Ab")
                nc.vector.tensor_copy(Ab[:], Ab_p[:])
                Bb = tmpool.tile([P, S], fp32, name=f"{name}_Bb")
                nc.vector.tensor_copy(Bb[:], Bb_p[:])
                h = hpool.tile([P, NT, S], out_dt, name=f"{name}_h")
                for t in range(NT):
                    tmp = tmpool.tile([P, S], fp32, name=f"{name}_t0_{t}",
                                      tag="ln_t0", bufs=3)
                    nc.vector.tensor_tensor(tmp[:], src[:, t, :], Ab[:],
                                            op=ALU.mult)
                    nc.vector.tensor_tensor(h[:, t, :], tmp[:], Bb[:],
                                            op=ALU.subtract)
                return h

            def accum(psum, src, wname, j, nt, tchunk, wpool, wbufs, fp8,
                      tag):
                """psum += sum_t W^T(t,j).T @ src[:,t,:]; fp8 uses DoubleRow
                over k-tile pairs."""
                nchunk = nt // tchunk
                wdt = f8 if fp8 else bf16
                for ci in range(nchunk):
                    wt = wpool.tile([P, tchunk, P], wdt, tag=tag,
                                    name=f"w_{wname}_{j}_{ci}", bufs=wbufs)
                    nc.sync.dma_start(
                        wt[:],
                        wd[wname].ap()[j, :,
                                       ci * tchunk:(ci + 1) * tchunk, :])
                    if fp8:
                        for ti in range(0, tchunk, 2):
                            t = ci * tchunk + ti
                            nc.tensor.matmul(psum[:], wt[:, ti:ti + 2, :],
                                             src[:, t:t + 2, :],
                                             start=(t == 0),
                                             stop=(t == nt - 2),
                                             perf_mode=DR)
                    else:
                        for ti in range(tchunk):
                            t = ci * tchunk + ti
                            nc.tensor.matmul(psum[:], wt[:, ti, :],
                                             src[:, t, :],
                                             start=(t == 0),
                                             stop=(t == nt - 1))

            def proj_gated(src_m, src_g, nt, nj, wname, wgname, bgname,
                           wpool, pspool, epilogue, gsc, m_fp8=True,
                           tchunk=None, wbufs=3, psbufs=3):
                """Gated projection in T-layout. Gate path is always fp8;
                main path fp8 iff m_fp8. sig = sigmoid(gate*gsc + bg)."""
                if tchunk is None:
                    tchunk = nt
                for j in range(nj):
                    main = pspool.tile([P, S], fp32, name=f"{wname}_m{j}",
                                       tag="pj_main", bufs=psbufs)
                    gate = pspool.tile([P, S], fp32, name=f"{wname}_g{j}",
                                       tag="pj_gate", bufs=psbufs)
                    accum(main, src_m, wname, j, nt, tchunk, wpool, wbufs,
                          m_fp8, "wmain")
                    accum(gate, src_g, wgname, j, nt, tchunk, wpool, wbufs,
                          True, "wgate")
                    sig = wpool.tile([P, S], bf16, tag="sig",
                                     name=f"sig_{wname}_{j}", bufs=3)
                    nc.scalar.activation(sig[:], gate[:], AF.Sigmoid,
                                         bias=bias[bgname][:, j:j + 1],
                                         scale=gsc)
                    epilogue(j, main, sig)

            DQ = [nc.sync, nc.gpsimd, nc.scalar]

            # x2 outlives phases A-C (used by LN2 + MLP residual)
            with tc.tile_pool(name="x2p", bufs=1) as x2p:
              with tc.tile_pool(name="xt", bufs=1) as xtp:
                xt = xtp.tile([P, NT, S], fp32)
                xT_v = xT_d.ap().rearrange("(t p) s -> t p s", p=P)
                for t in range(NT):
                    DQ[t % 3].dma_start(xt[:, t, :], xT_v[t])

                vN_bounce = dramp.tile([S, D], f8)
                k_bounce = dramp.tile([D, S], f8)
                vgN = dramp.tile([GROUP * S, D], f8)
                kg = dramp.tile([GROUP * D, S], f8)

                with tc.tile_pool(name="yp", bufs=1) as ypool:
                  with (
                      tc.tile_pool(name="qp", bufs=1) as qpool,
                      tc.tile_pool(name="kstream", bufs=1) as kpool,
                  ):
                    q = qpool.tile([P, NT, S], f8)
                    khs = {}

                    def load_Kh(hh, queue):
                        Kh = kpool.tile([P, NKB * P], f8, tag="Kh",
                                        name=f"Kh_{hh}", bufs=6)
                        for s_ in range(GROUP):
                            queue.dma_start(
                                Kh[:, s_ * S:(s_ + 1) * S],
                                kg[s_ * D + hh * P:
                                   s_ * D + (hh + 1) * P, :])
                        khs[hh] = Kh

                    with tc.tile_pool(name="hq", bufs=1) as hqp:
                        with (
                            tc.tile_pool(name="ln1tmp", bufs=1) as ln1tmp,
                            tc.tile_pool(name="ln1ps", bufs=1,
                                         space="PSUM") as ln1ps,
                        ):
                            h1 = ln_T(xt, hqp, ln1tmp, ln1ps, "ln1")

                        # ---- K projection (T-layout) + AllGather ----
                        with (
                            tc.tile_pool(name="wprojk", bufs=1) as wpoolk,
                            tc.tile_pool(name="pjpsk", bufs=1,
                                         space="PSUM") as pjpsk,
                        ):
                            def k_epi(j, main, sig):
                                kv = wpoolk.tile([P, S], f8, tag="kv_out",
                                                 name=f"kv_k_{j}", bufs=3)
                                nc.vector.scalar_tensor_tensor(
                                    kv[:], main[:], bias["bk"][:, j:j + 1],
                                    sig[:], op0=ALU.add, op1=ALU.mult)
                                nc.scalar.dma_start(
                                    k_bounce[j * P:(j + 1) * P, :], kv[:])

                            proj_gated(h1, h1, NT, NT, "Wk", "Wgk", "bgk",
                                       wpoolk, pjpsk, k_epi, 1.0 / WS)

                            nc.gpsimd.collective_compute(
                                "AllGather", ALU.bypass, ins=[k_bounce[:]],
                                outs=[kg[:]], replica_groups=RG)

                        # Preload first heads' K right behind the gather
                        # (vector queue: transfers start as AG_K lands).
                        for hh in range(4):
                            load_Kh(hh, nc.scalar)

                        # ---- V projection, N-layout + AllGather ----
                        with (
                            tc.tile_pool(name="wv", bufs=1) as wvp,
                            tc.tile_pool(name="vps", bufs=1,
                                         space="PSUM") as vps,
                        ):
                            TC = NT // 2
                            for n in range(4):
                                vmain = [vps.tile([P, S], fp32,
                                                  tag="v_main", bufs=4,
                                                  name=f"vm_{n}_{m}")
                                         for m in range(4)]
                                vgate = [vps.tile([P, S], fp32,
                                                  tag="v_gate", bufs=4,
                                                  name=f"vg_{n}_{m}")
                                         for m in range(4)]
                                for ci in range(2):
                                    wvt = wvp.tile([P, TC, 4 * P], f8,
                                                   tag="wv", bufs=2,
                                                   name=f"wv_{n}_{ci}")
                                    wgvt = wvp.tile([P, TC, 4 * P], f8,
                                                    tag="wgv", bufs=2,
                                                    name=f"wgv_{n}_{ci}")
                                    for ti in range(TC):
                                        t = ci * TC + ti
                                        nc.sync.dma_start(
                                            wvt[:, ti, :],
                                            wd["WvT"].ap()[t * P:(t + 1) * P,
                                                           n * S:(n + 1) * S])
                                        nc.sync.dma_start(
                                            wgvt[:, ti, :],
                                            wd["WgvT"].ap()[
                                                t * P:(t + 1) * P,
                                                n * S:(n + 1) * S])
                                    for m in range(4):
                                        for ti in range(0, TC, 2):
                                            t = ci * TC + ti
                                            nc.tensor.matmul(
                                                vmain[m][:],
                                                h1[:, t:t + 2,
                                                   m * P:(m + 1) * P],
                                                wvt[:, ti:ti + 2, :],
                                                start=(t == 0), stop=False,
                                                perf_mode=DR)
                                        for ti in range(0, TC, 2):
                                            t = ci * TC + ti
                                            nc.tensor.matmul(
                                                vgate[m][:],
                                                h1[:, t:t + 2,
                                                   m * P:(m + 1) * P],
                                                wgvt[:, ti:ti + 2, :],
                                                start=(t == 0), stop=False,
                                                perf_mode=DR)
                                for m in range(4):
                                    nc.tensor.matmul(
                                        vmain[m][:], ones_row[:],
                                        bvrow[:, n * S:(n + 1) * S],
                                        start=False, stop=True)
                                    nc.tensor.matmul(
                                        vgate[m][:], ones_row[:],
                                        bgvrow[:, n * S:(n + 1) * S],
                                        start=False, stop=True)
                                    vsig = wvp.tile([P, S], bf16,
                                                    tag="vsig", bufs=3,
                                                    name=f"vsig_{n}_{m}")
                                    nc.scalar.activation(vsig[:],
                                                         vgate[m][:],
                                                         AF.Sigmoid,
                                                         scale=1.0 / WS)
                                    vout = wvp.tile([P, S], f8,
                                                    tag="vout", bufs=3,
                                                    name=f"vout_{n}_{m}")
                                    nc.vector.tensor_tensor(
                                        vout[:], vmain[m][:], vsig[:],
                                        op=ALU.mult)
                                    nc.scalar.dma_start(
                                        vN_bounce[m * P:(m + 1) * P,
                                                  n * S:(n + 1) * S],
                                        vout[:])

                        nc.gpsimd.collective_compute(
                            "AllGather", ALU.bypass, ins=[vN_bounce[:]],
                            outs=[vgN[:]], replica_groups=RG)

                        # ---- Q projection ----
                        with (
                            tc.tile_pool(name="wprojq", bufs=1) as wpoolq,
                            tc.tile_pool(name="pjpsq", bufs=1,
                                         space="PSUM") as pjpsq,
                        ):
                            def q_epi(j, main, sig):
                                nc.vector.scalar_tensor_tensor(
                                    q[:, j, :], main[:],
                                    bias["bq"][:, j:j + 1],
                                    sig[:], op0=ALU.add, op1=ALU.mult)

                            proj_gated(h1, h1, NT, NT, "Wq", "Wgq", "bgq",
                                       wpoolq, pjpsq, q_epi, 1.0 / WS)

                    # ---- phase B: attention ----
                    with (
                        tc.tile_pool(name="vres", bufs=1) as vresp,
                        tc.tile_pool(name="apool", bufs=4) as apool,
                        tc.tile_pool(name="atps", bufs=1,
                                     space="PSUM") as atps,
                    ):
                        y = ypool.tile([P, NT, S], f8)
                        # V resident [k-part, kb, d]: gathered fp8, loaded
                        # on the vector queue and upcast to bf16 for PV.
                        Vt = vresp.tile([P, NKB, D], bf16)
                        for kb in range(NKB):
                            v8 = vresp.tile([P, D], f8, tag="Vt8",
                                            name=f"Vt8_{kb}", bufs=3)
                            nc.scalar.dma_start(
                                v8[:], vgN[kb * P:(kb + 1) * P, :])
                            nc.vector.tensor_copy(Vt[:, kb, :], v8[:])

                        head_state = {}

                        def finalize_head(h, Zp_h, Yp_h):
                            urow = rows.tile([1, S], fp32, name=f"u_{h}",
                                             tag="urow", bufs=2)
                            nc.vector.reciprocal_approx_fast(urow[:],
                                                             Zp_h[:])
                            ubf = rows.tile([1, S], bf16, name=f"ubf_{h}",
                                            tag="ubf", bufs=2)
                            nc.vector.tensor_copy(ubf[:], urow[:])
                            Up = atps.tile([P, S], fp32, name=f"Up_{h}",
                                           tag="logits", bufs=4)
                            nc.tensor.matmul(Up[:], ones_row[:], ubf[:])
                            Us = apool.tile([P, S], bf16, tag="Us",
                                            name=f"Us_{h}")
                            nc.vector.tensor_copy(Us[:], Up[:])
                            nc.vector.tensor_tensor(y[:, h, :], Yp_h[:],
                                                    Us[:], op=ALU.mult)

                        for hh in range(H):
                            if hh not in khs:
                                load_Kh(hh, nc.scalar)
                            Kh = khs[hh]
                            Yp = atps.tile([P, S], fp32, name=f"Y_{hh}",
                                           tag="Yp", bufs=2)
                            ats = {}
                            zsum = {}

                            def do_L(kb, hh=hh, Kh=Kh, ats=ats):
                                Lp = atps.tile([P, S], fp32,
                                               name=f"L_{hh}_{kb}",
                                               tag="logits", bufs=4)
                                nc.tensor.matmul(
                                    Lp[:], Kh[:, kb * P:(kb + 1) * P],
                                    q[:, hh, :])
                                At = apool.tile([P, S], bf16, tag="At",
                                                name=f"At_{hh}_{kb}",
                                                bufs=6)
                                nc.scalar.activation(
                                    At[:], Lp[:], AF.Exp,
                                    scale=ISCALE / (WS * WS))
                                ats[kb] = At

                            do_L(0)
                            do_L(1)
                            for kb in range(NKB):
                                if kb + 2 < NKB:
                                    do_L(kb + 2)
                                # softmax denominator on DVE (frees PE)
                                if kb == 1:
                                    zs = apool.tile([P, S], fp32,
                                                    tag="Ssum", bufs=3,
                                                    name=f"Zs_{hh}_1")
                                    nc.vector.tensor_tensor(
                                        zs[:], ats[0][:], ats[1][:],
                                        op=ALU.add)
                                    zsum[0] = zs
                                elif kb > 1:
                                    zs = apool.tile([P, S], fp32,
                                                    tag="Ssum", bufs=3,
                                                    name=f"Zs_{hh}_{kb}")
                                    nc.vector.tensor_tensor(
                                        zs[:], zsum[0][:], ats[kb][:],
                                        op=ALU.add)
                                    zsum[0] = zs
                                nc.tensor.matmul(
                                    Yp[:],
                                    Vt[:, kb, hh * P:(hh + 1) * P],
                                    ats[kb][:],
                                    start=(kb == 0),
                                    stop=(kb == NKB - 1))
                                if kb == 3 and hh > 0:
                                    finalize_head(hh - 1,
                                                  *head_state[hh - 1])
                            Zp = atps.tile([1, S], fp32, name=f"Z_{hh}",
                                           tag="Zp", bufs=2)
                            nc.tensor.matmul(Zp[:], ones_col_f[:],
                                             zsum[0][:])
                            head_state[hh] = (Zp, Yp)
                        finalize_head(H - 1, *head_state[H - 1])

                  # ---- phase C: o-proj + residual ----
                  x2 = x2p.tile([P, NT, S], fp32, name="x2")
                  with (
                      tc.tile_pool(name="wproj2", bufs=1) as wpool2,
                      tc.tile_pool(name="pj2ps", bufs=1,
                                   space="PSUM") as pj2ps,
                  ):
                      def o_epi(j, main, sig):
                          tmp = wpool2.tile([P, S], fp32, tag="o_tmp",
                                            name=f"o_tmp_{j}", bufs=3)
                          nc.vector.scalar_tensor_tensor(
                              tmp[:], main[:], bias["bo"][:, j:j + 1],
                              sig[:], op0=ALU.add, op1=ALU.mult)
                          nc.vector.scalar_tensor_tensor(
                              x2[:, j, :], tmp[:], invo[:],
                              xt[:, j, :], op0=ALU.mult, op1=ALU.add)

                      proj_gated(y, y, NT, NT, "Wo", "Wgo", "bgo",
                                 wpool2, pj2ps, o_epi, 1.0 / (WS * WS))

              # ---- phase D: LN2 + MLP ----
              with (
                  tc.tile_pool(name="midp", bufs=1) as midp,
                  tc.tile_pool(name="mid8p", bufs=1) as mid8p,
              ):
                  mid = midp.tile([P, NF, S], bf16)
                  mid8 = mid8p.tile([P, NF, S], f8)
                  with tc.tile_pool(name="h2p", bufs=1) as h2p:
                      with (
                          tc.tile_pool(name="ln2tmp", bufs=1) as ln2tmp,
                          tc.tile_pool(name="ln2ps", bufs=1,
                                       space="PSUM") as ln2ps,
                      ):
                          h2 = ln_T(x2, h2p, ln2tmp, ln2ps, "ln2",
                                    out_dt=in_main_dt)
                          if FP8_IN_MAIN:
                              h2m = h2g = h2
                          else:
                              h2g8 = h2p.tile([P, NT, S], f8, name="h2g8")
                              for t in range(NT):
                                  nc.vector.tensor_copy(h2g8[:, t, :],
                                                        h2[:, t, :])
                              h2m, h2g = h2, h2g8

                      with (
                          tc.tile_pool(name="wmlp1", bufs=1) as wm1,
                          tc.tile_pool(name="m1ps", bufs=1,
                                       space="PSUM") as m1ps,
                      ):
                          def mid_epi(j, main, sig):
                              tmp = wm1.tile([P, S], fp32, tag="mid_tmp",
                                             name=f"mid_tmp_{j}", bufs=3)
                              nc.vector.scalar_tensor_tensor(
                                  tmp[:], main[:],
                                  bias["bin"][:, j:j + 1], sig[:],
                                  op0=ALU.add, op1=ALU.mult)
                              nc.scalar.activation(
                                  mid[:, j, :], tmp[:], AF.Gelu,
                                  scale=(1.0 / WS if FP8_IN_MAIN else 1.0))
                              nc.vector.tensor_copy(mid8[:, j, :],
                                                    mid[:, j, :])

                          proj_gated(h2m, h2g, NT, NF, "Win", "Wgin",
                                     "bgin", wm1, m1ps, mid_epi, 1.0 / WS,
                                     m_fp8=FP8_IN_MAIN)

                  with (
                      tc.tile_pool(name="wmlp2", bufs=1) as wm2,
                      tc.tile_pool(name="m2ps", bufs=1,
                                   space="PSUM") as m2ps,
                  ):
                      def out_epi(j, main, sig):
                          tmp = wm2.tile([P, S], fp32, tag="out_tmp",
                                         name=f"out_tmp_{j}", bufs=3)
                          nc.vector.scalar_tensor_tensor(
                              tmp[:], main[:], bias["bout"][:, j:j + 1],
                              sig[:], op0=ALU.add, op1=ALU.mult)
                          outf = wm2.tile([P, S], fp32, tag="out_f",
                                          name=f"out_f_{j}", bufs=3)
                          nc.vector.tensor_tensor(outf[:], tmp[:],
                                                  x2[:, j, :], op=ALU.add)
                          DQ[j % 3].dma_start(
                              out_d.ap()[j * P:(j + 1) * P, :], outf[:])

                      proj_gated(mid, mid8, NF, NT, "Wout", "Wgout",
                                 "bgout", wm2, m2ps, out_epi, 1.0 / WS,
                                 m_fp8=False, tchunk=32, wbufs=2)

    nc.compile()
    return nc


def _prep_shared_inputs(inputs):
    f32 = np.float32
    g1 = np.asarray(inputs["ln1_g"], f32)
    b1 = np.asarray(inputs["ln1_b"], f32)
    g2 = np.asarray(inputs["ln2_g"], f32)
    b2 = np.asarray(inputs["ln2_b"], f32)

    # Fold the LN affine into the consumer weights/biases:
    #   h_affine = h_norm * g + b  =>  W' = W*g[None,:], b' = b_proj + W@b
    def fold(wn, bn, g, bln):
        W = np.asarray(inputs[wn], f32)
        bb = np.asarray(inputs[bn], f32)
        return W * g[None, :], bb + W @ bln

    m = {}
    folded_b = {}
    for nm, wn, bn, g, bln in (
            ("Wq", "W_q", "b_q", g1, b1), ("Wgq", "Wg_q", "bg_q", g1, b1),
            ("Wk", "W_k", "b_k", g1, b1), ("Wgk", "Wg_k", "bg_k", g1, b1),
            ("Win", "W_in", "b_in", g2, b2),
            ("Wgin", "Wg_in", "bg_in", g2, b2)):
        W, bb = fold(wn, bn, g, bln)
        if nm == "Win" and not FP8_IN_MAIN:
            m[nm] = _w_tiled_bf(W)
        else:
            m[nm] = _w_tiled_f8(W)
        folded_b[bn] = bb
    for nm, wn in (("Wo", "W_o"), ("Wgo", "Wg_o"), ("Wgout", "Wg_out")):
        m[nm] = _w_tiled_f8(np.asarray(inputs[wn], f32))
    m["Wout"] = _w_tiled_bf(np.asarray(inputs["W_out"], f32))
    Wv, bv = fold("W_v", "b_v", g1, b1)
    Wgv, bgv = fold("Wg_v", "bg_v", g1, b1)
    m["WvT"] = np.ascontiguousarray(
        np.clip(Wv.T * WS, -240, 240).astype(_F8))
    m["WgvT"] = np.ascontiguousarray(
        np.clip(Wgv.T * WS, -240, 240).astype(_F8))
    m["bvrow"] = (bv * WS).astype(_BF).reshape(1, D)
    m["bgvrow"] = (bgv * WS).astype(_BF).reshape(1, D)
    main_b_scale = {"bq": WS, "bk": WS, "bo": WS * WS,
                    "bin": (WS if FP8_IN_MAIN else 1.0), "bout": 1.0}
    for nm, bn in (("bq", "b_q"), ("bgq", "bg_q"), ("bk", "b_k"),
                   ("bgk", "bg_k"), ("bo", "b_o"), ("bgo", "bg_o"),
                   ("bin", "b_in"), ("bgin", "bg_in"), ("bout", "b_out"),
                   ("bgout", "bg_out")):
        bb = folded_b.get(bn, None)
        if bb is None:
            bb = np.asarray(inputs[bn], f32)
        m[nm] = _b_cols(bb, main_b_scale.get(nm, 1.0))
    return m


def _install_trace_shim():
    """Provide antenv.axon_hooks (NTFF profiling) if the image lacks it."""
    import contextlib
    import ctypes
    import types

    try:
        import antenv.axon_hooks  # noqa: F401
        return
    except ImportError:
        pass
    try:
        import antenv
    except ImportError:
        return
    so_path = "/opt/axon/libaxon_pjrt.so"
    try:
        lib = ctypes.CDLL(so_path)
    except OSError:
        return
    if not hasattr(lib, "axon_start_nrt_profile"):
        return
    lib.axon_start_nrt_profile.argtypes = [ctypes.POINTER(ctypes.c_int64),
                                           ctypes.c_size_t]
    lib.axon_start_nrt_profile.restype = ctypes.c_int64
    lib.axon_stop_nrt_profile.argtypes = [ctypes.c_char_p]
    lib.axon_stop_nrt_profile.restype = ctypes.c_int64

    @contextlib.contextmanager
    def hook(output_dir, device_ids):
        import jax

        jax.devices()
        if device_ids:
            ids = (ctypes.c_int64 * len(device_ids))(*device_ids)
            rc = lib.axon_start_nrt_profile(ids, len(device_ids))
        else:
            rc = lib.axon_start_nrt_profile(None, 0)
        if rc != 0:
            raise RuntimeError(f"axon_start_nrt_profile rc={rc}")
        try:
            yield
        finally:
            n = lib.axon_stop_nrt_profile(str(output_dir).encode())
            print(f"profile: {n} ntff file(s) in {output_dir}",
                  file=sys.stderr)

    mod = types.ModuleType("antenv.axon_hooks")
    mod.get_axon_ntff_profile_hook = lambda: hook
    mod.set_axon_ntff_profile_hook = lambda h: None
    sys.modules["antenv.axon_hooks"] = mod
    antenv.axon_hooks = mod


LAST_RESULTS = None


def kernel(_trace=False, **inputs):
    global _COMPILED, LAST_RESULTS
    from concourse import bass_utils

    if _trace:
        _install_trace_shim()

    if _COMPILED is None:
        _COMPILED = _build()
    nc = _COMPILED

    shared = _prep_shared_inputs(inputs)
    x = np.asarray(inputs["x"], dtype=np.float32)  # [B, T, D]
    in_maps = []
    for c in range(N_CORES):
        g, s = divmod(c, GROUP)
        xT_c = np.ascontiguousarray(x[g, s * S:(s + 1) * S, :].T)
        m = dict(shared)
        m["xT"] = xT_c
        in_maps.append(m)

    LAST_RESULTS = bass_utils.run_bass_kernel_spmd(
        nc, in_maps, core_ids=list(range(N_CORES)), trace=_trace)

    out = np.empty((B, T, D), dtype=np.float32)
    for c in range(N_CORES):
        g, s = divmod(c, GROUP)
        out[g, s * S:(s + 1) * S, :] = LAST_RESULTS.results[c]["outT"].T
    return out


# revision 16
# speedup vs baseline: 1.0501x; 1.0441x over previous
"""Trainium2 Bass kernel for nn_DGEBlock (dense transformer block with
MoE-gated linears), distributed over 8 NeuronCores.

Sharding: data-parallel over batch (2 groups of 4 cores) x sequence-parallel
over tokens within each batch (512 tokens per core). Weights are replicated
(host pre-tiled); activations live feature-major ("T-layout": [d, tok]) in
SBUF so projections are lhsT=W^T-tile @ rhs=activation with no activation
transposes. V is projected in token-major (N-)layout directly so attention's
PV matmuls need no transposes either.

FP8 (e4m3) with DoubleRow double-pumping on the TensorEngine for the
projection matmuls: q/k/v/o main+gate, mlp_in main+gate, mlp_out gate.
Weights are pre-scaled x16 on the host so their values sit in the fp8
normal range; descaling is deferred into existing epilogue scalars. The
LN affine (gain/shift) is folded into the downstream weights host-side.
mlp_out's main matmul stays bf16 (its quantization error dominates the
output). Attention: QK in plain fp8, exp/PV in bf16 (exp range exceeds
fp8), softmax denominator accumulated on the Vector engine.

Scheduling: weight-stream and PSUM pools are opened BEFORE the LN tmp
pools so weight DMA issue never waits for LN to finish; collectives are
gathered fp8 and sequenced on the gpsimd queue as [AG_V, Vt8-loads,
AG_K, Kh-preloads] so both hide under the K/Q projections; per-tile
weight loads are single rearranged-AP DMA descriptors (the sequencer
pays ~600ns per issue); x/out tiles round-robin across the three
DMA-capable queues.
"""

import sys

for _p in ("/opt/trn_rl_repo",):
    if _p not in sys.path:
        sys.path.append(_p)

import numpy as np
import ml_dtypes

# ---------------------------------------------------------------- constants
B = 2
T = 2048
D = 2048
H = 16
HD = 128
FF = 4 * D  # 8192
EPS = 1e-5

N_CORES = 8
GROUP = 4  # cores per batch group (sequence-parallel degree)
S = T // GROUP  # tokens per core = 512
P = 128
NT = D // P  # 16 feature tiles
NF = FF // P  # 64 hidden tiles
NKB = T // P  # 16 key blocks per batch
ISCALE = 1.0 / float(np.sqrt(HD))

WS = 16.0  # fp8 weight pre-scale
FP8_IN_MAIN = True  # mlp_in main matmul in fp8 (False -> bf16 fallback)

RG = [[0, 1, 2, 3], [4, 5, 6, 7]]

_BF = ml_dtypes.bfloat16
_F8 = ml_dtypes.float8_e4m3

_COMPILED = None


# ------------------------------------------------------------- host prep
def _tile4(W):
    """W [dout, din] -> [nj, 128, nt, 128] such that
    out[j, p, t, jc] == W[j*128+jc, t*128+p]  (= W^T tile (t, j))."""
    dout, din = W.shape
    nj, nt = dout // P, din // P
    return W.reshape(nj, P, nt, P).transpose(0, 3, 2, 1)


def _w_tiled_bf(W):
    return np.ascontiguousarray(_tile4(W).astype(_BF))


def _w_tiled_f8(W):
    return np.ascontiguousarray(
        np.clip(_tile4(W) * WS, -240, 240).astype(_F8)
    )


def _b_cols(b, scale=1.0):
    """b [dout] -> [128, nj] fp32: column j holds b[j*128:(j+1)*128]."""
    nj = b.shape[0] // P
    return np.ascontiguousarray((b * scale).reshape(nj, P).T.astype(np.float32))


# ------------------------------------------------------------- device build
def _build():
    from concourse import bacc, tile, mybir

    fp32 = mybir.dt.float32
    bf16 = mybir.dt.bfloat16
    f8 = mybir.dt.float8e4
    AF = mybir.ActivationFunctionType
    ALU = mybir.AluOpType
    DR = mybir.MatmulPerfMode.DoubleRow

    in_main_dt = f8 if FP8_IN_MAIN else bf16

    nc = bacc.Bacc("TRN2", target_bir_lowering=False, debug=False,
                   num_devices=N_CORES)

    # ---- I/O tensors
    xT_d = nc.dram_tensor("xT", [D, S], fp32, kind="ExternalInput")
    wd = {}
    for nm in ("Wq", "Wgq", "Wk", "Wgk", "Wo", "Wgo"):
        wd[nm] = nc.dram_tensor(nm, [NT, P, NT, P], f8, kind="ExternalInput")
    wd["Win"] = nc.dram_tensor("Win", [NF, P, NT, P], in_main_dt,
                               kind="ExternalInput")
    wd["Wgin"] = nc.dram_tensor("Wgin", [NF, P, NT, P], f8,
                                kind="ExternalInput")
    wd["Wout"] = nc.dram_tensor("Wout", [NT, P, NF, P], bf16,
                                kind="ExternalInput")
    wd["Wgout"] = nc.dram_tensor("Wgout", [NT, P, NF, P], f8,
                                 kind="ExternalInput")
    # V projection runs in N-layout: plain W^T [din, dout] + bias rows
    wd["WvT"] = nc.dram_tensor("WvT", [D, D], f8, kind="ExternalInput")
    wd["WgvT"] = nc.dram_tensor("WgvT", [D, D], f8, kind="ExternalInput")
    bvrow_d = nc.dram_tensor("bvrow", [1, D], bf16, kind="ExternalInput")
    bgvrow_d = nc.dram_tensor("bgvrow", [1, D], bf16, kind="ExternalInput")
    bd = {}
    for nm in ("bq", "bgq", "bk", "bgk", "bo", "bgo", "bout", "bgout"):
        bd[nm] = nc.dram_tensor(nm, [P, NT], fp32, kind="ExternalInput")
    for nm in ("bin", "bgin"):
        bd[nm] = nc.dram_tensor(nm, [P, NF], fp32, kind="ExternalInput")
    out_d = nc.dram_tensor("outT", [D, S], fp32, kind="ExternalOutput")

    with tile.TileContext(nc) as tc:
        with (
            tc.tile_pool(name="const", bufs=1) as constp,
            tc.tile_pool(name="bias", bufs=1) as biasp,
            tc.tile_pool(name="rows", bufs=1) as rows,
            tc.tile_pool(name="dram", bufs=1, space="DRAM") as dramp,
        ):
            DQ = [nc.sync, nc.gpsimd, nc.scalar]

            ones_col = constp.tile([P, 1], bf16)
            nc.vector.memset(ones_col[:], 1.0)
            ones_col_f = constp.tile([P, 1], fp32)
            nc.vector.memset(ones_col_f[:], 1.0)
            ones_row = constp.tile([1, P], bf16)
            nc.vector.memset(ones_row[:], 1.0)
            eps_t = constp.tile([1, 1], fp32)
            nc.vector.memset(eps_t[:], EPS)
            invo = constp.tile([P, 1], fp32)
            nc.vector.memset(invo[:], 1.0 / (WS * WS))

            bias = {}

            def load_consts():
                bvrow = constp.tile([1, D], bf16)
                nc.scalar.dma_start(bvrow[:], bvrow_d.ap())
                bgvrow = constp.tile([1, D], bf16)
                nc.scalar.dma_start(bgvrow[:], bgvrow_d.ap())
                for nm in bd:
                    ncols = NF if nm in ("bin", "bgin") else NT
                    btile = biasp.tile([P, ncols], fp32, name=f"bias_{nm}")
                    nc.scalar.dma_start(btile[:], bd[nm].ap())
                    bias[nm] = btile
                return bvrow, bgvrow

            # ---------- helpers ----------
            def ln_T(get_src, hpool, tmpool, psln, name, out_dt=f8):
                """LayerNorm (stats+normalize only; affine folded into
                consumer weights host-side). get_src(t, pass_idx) yields
                [128, S] fp32 tiles -> out_dt [128, NT, S]. The per-tile
                normalize runs the multiply on GpSimd and the subtract
                (+fp8 cast) on DVE."""
                S1 = psln.tile([1, S], fp32, name=f"{name}_S1",
                               tag="lnS", bufs=2)
                S2 = psln.tile([1, S], fp32, name=f"{name}_S2",
                               tag="lnS", bufs=2)
                for t in range(NT):
                    srct = get_src(t, 0)
                    xbf = tmpool.tile([P, S], bf16, name=f"{name}_xbf_{t}",
                                      tag="ln_xbf", bufs=2)
                    nc.vector.tensor_copy(xbf[:], srct)
                    sq = tmpool.tile([P, S], bf16, name=f"{name}_sq_{t}",
                                     tag="ln_sq", bufs=2)
                    nc.scalar.activation(sq[:], srct, AF.Square)
                    nc.tensor.matmul(S1[:], ones_col[:], xbf[:],
                                     start=(t == 0), stop=(t == NT - 1))
                    nc.tensor.matmul(S2[:], ones_col[:], sq[:],
                                     start=(t == 0), stop=(t == NT - 1))

                def row(nm, dt=fp32):
                    return rows.tile([1, S], dt, name=f"{name}_{nm}",
                                     tag=f"ln_{nm}")

                mean = row("mean")
                nc.vector.tensor_scalar_mul(mean[:], S1[:], 1.0 / D)
                m2 = row("m2")
                nc.vector.tensor_scalar_mul(m2[:], S2[:], 1.0 / D)
                msq = row("msq")
                nc.vector.tensor_tensor(msq[:], mean[:], mean[:],
                                        op=ALU.mult)
                var = row("var")
                nc.vector.tensor_tensor(var[:], m2[:], msq[:],
                                        op=ALU.subtract)
                std = row("std")
                nc.scalar.activation(std[:], var[:], AF.Sqrt, bias=eps_t[:])
                rstd = row("rstd")
                nc.vector.reciprocal_approx_fast(rstd[:], std[:])
                rstd_bf = row("rstdbf", bf16)
                nc.vector.tensor_copy(rstd_bf[:], rstd[:])
                mr_bf = row("mrbf", bf16)
                nc.vector.tensor_tensor(mr_bf[:], mean[:], rstd[:],
                                        op=ALU.mult)
                Ab_p = psln.tile([P, S], fp32, name=f"{name}_Abp",
                                 tag="pj_main", bufs=3)
                nc.tensor.matmul(Ab_p[:], ones_row[:], rstd_bf[:])
                Bb_p = psln.tile([P, S], fp32, name=f"{name}_Bbp",
                                 tag="pj_gate", bufs=3)
                nc.tensor.matmul(Bb_p[:], ones_row[:], mr_bf[:])
                Ab = tmpool.tile([P, S], fp32, name=f"{name}_Ab")
                nc.vector.tensor_copy(Ab[:], Ab_p[:])
                Bb = tmpool.tile([P, S], fp32, name=f"{name}_Bb")
                nc.vector.tensor_copy(Bb[:], Bb_p[:])
                h = hpool.tile([P, NT, S], out_dt, name=f"{name}_h")
                for t in range(NT):
                    srct = get_src(t, 1)
                    tmp = tmpool.tile([P, S], fp32, name=f"{name}_t0_{t}",
                                      tag="ln_t0", bufs=3)
                    nc.gpsimd.tensor_tensor(tmp[:], srct, Ab[:],
                                            op=ALU.mult)
                    nc.vector.tensor_tensor(h[:, t, :], tmp[:], Bb[:],
                                            op=ALU.subtract)
                return h

            def accum(psum, src, wname, j, nt, tchunk, wpool, wbufs, fp8,
                      tag):
                """psum += sum_t W^T(t,j).T @ src[:,t,:] (DoubleRow pairs
                when fp8). One DMA descriptor per weight tile."""
                nchunk = nt // tchunk
                wdt = f8 if fp8 else bf16
                for ci in range(nchunk):
                    wt = wpool.tile([P, tchunk, P], wdt, tag=tag,
                                    name=f"w_{wname}_{j}_{ci}", bufs=wbufs)
                    nc.sync.dma_start(
                        wt[:],
                        wd[wname].ap()[j, :,
                                       ci * tchunk:(ci + 1) * tchunk, :])
                    if fp8:
                        for ti in range(0, tchunk, 2):
                            t = ci * tchunk + ti
                            nc.tensor.matmul(psum[:], wt[:, ti:ti + 2, :],
                                             src[:, t:t + 2, :],
                                             start=(t == 0),
                                             stop=(t == nt - 2),
                                             perf_mode=DR)
                    else:
                        for ti in range(tchunk):
                            t = ci * tchunk + ti
                            nc.tensor.matmul(psum[:], wt[:, ti, :],
                                             src[:, t, :],
                                             start=(t == 0),
                                             stop=(t == nt - 1))

            def proj_gated(src_m, src_g, nt, nj, wname, wgname, bgname,
                           wpool, pspool, epilogue, gsc, m_fp8=True,
                           tchunk=None, wbufs=3):
                """Gated projection in T-layout. Gate path is always fp8;
                main path fp8 iff m_fp8."""
                if tchunk is None:
                    tchunk = nt
                for j in range(nj):
                    main = pspool.tile([P, S], fp32, name=f"{wname}_m{j}",
                                       tag="pj_main", bufs=3)
                    gate = pspool.tile([P, S], fp32, name=f"{wname}_g{j}",
                                       tag="pj_gate", bufs=3)
                    accum(main, src_m, wname, j, nt, tchunk, wpool, wbufs,
                          m_fp8, "wmain")
                    accum(gate, src_g, wgname, j, nt, tchunk, wpool, wbufs,
                          True, "wgate")
                    sig = wpool.tile([P, S], bf16, tag="sig",
                                     name=f"sig_{wname}_{j}", bufs=3)
                    nc.scalar.activation(sig[:], gate[:], AF.Sigmoid,
                                         bias=bias[bgname][:, j:j + 1],
                                         scale=gsc)
                    epilogue(j, main, sig)

            with tc.tile_pool(name="x2p", bufs=1) as x2p:
              with tc.tile_pool(name="xlnp", bufs=1) as xlnp:
                xT_v = xT_d.ap().rearrange("(t p) s -> t p s", p=P)

                def x_src(t, pass_idx):
                    xa = xlnp.tile([P, S], fp32, tag=f"xln{pass_idx}",
                                   bufs=(3 if pass_idx == 0 else 4),
                                   name=f"x_{pass_idx}_{t}")
                    DQ[t % 3].dma_start(xa[:], xT_v[t])
                    return xa[:]

                bvrow, bgvrow = load_consts()

                vN_bounce = dramp.tile([S, D], f8)
                k_bounce = dramp.tile([D, S], f8)
                vgN = dramp.tile([GROUP * S, D], f8)
                kg = dramp.tile([GROUP * D, S], f8)
                kg_v = kg[:, :].rearrange("(s d) c -> d s c", d=D)
                vgN_v = vgN[:, :].rearrange("(kb p) c -> p kb c", p=P)
                WvT_v = wd["WvT"].ap().rearrange("(t p) c -> p t c", p=P)
                WgvT_v = wd["WgvT"].ap().rearrange("(t p) c -> p t c", p=P)

                with tc.tile_pool(name="yp", bufs=1) as ypool:
                  with tc.tile_pool(name="qp", bufs=1) as qpool, \
                       tc.tile_pool(name="kstream", bufs=1) as kpool, \
                       tc.tile_pool(name="vres", bufs=1) as vresp:
                    q = qpool.tile([P, NT, S], f8)
                    Vt = vresp.tile([P, NKB, D], f8)
                    khs = {}

                    def load_Kh(hh):
                        Kh = kpool.tile([P, GROUP, S], f8, tag="Kh",
                                        name=f"Kh_{hh}", bufs=6)
                        nc.gpsimd.dma_start(Kh[:],
                                            kg_v[hh * P:(hh + 1) * P])
                        khs[hh] = Kh

                    with (
                        tc.tile_pool(name="wsA", bufs=1) as wsA,
                        tc.tile_pool(name="pjpsA", bufs=1,
                                     space="PSUM") as pjpsA,
                    ):
                        with tc.tile_pool(name="hq", bufs=1) as hqp:
                            with tc.tile_pool(name="ln1tmp",
                                              bufs=1) as ln1tmp:
                                h1 = ln_T(x_src, hqp, ln1tmp, pjpsA,
                                          "ln1")

                            # ---- V projection (N-layout, m-outer) ----
                            TC = NT // 2
                            for n in range(4):
                                wvts = []
                                for ci in range(2):
                                    wvt = wsA.tile([P, TC, 4 * P], f8,
                                                   tag="wv", bufs=2,
                                                   name=f"wv_{n}_{ci}")
                                    nc.sync.dma_start(
                                        wvt[:],
                                        WvT_v[:, ci * TC:(ci + 1) * TC,
                                              n * S:(n + 1) * S])
                                    wgvt = wsA.tile([P, TC, 4 * P], f8,
                                                    tag="wgv", bufs=2,
                                                    name=f"wgv_{n}_{ci}")
                                    nc.sync.dma_start(
                                        wgvt[:],
                                        WgvT_v[:, ci * TC:(ci + 1) * TC,
                                               n * S:(n + 1) * S])
                                    wvts.append((wvt, wgvt))
                                for m in range(4):
                                    vmain = pjpsA.tile([P, S], fp32,
                                                       tag="pj_main",
                                                       bufs=3,
                                                       name=f"vm_{n}_{m}")
                                    vgate = pjpsA.tile([P, S], fp32,
                                                       tag="pj_gate",
                                                       bufs=3,
                                                       name=f"vg_{n}_{m}")
                                    for ci in range(2):
                                        wvt, wgvt = wvts[ci]
                                        for ti in range(0, TC, 2):
                                            t = ci * TC + ti
                                            nc.tensor.matmul(
                                                vmain[:],
                                                h1[:, t:t + 2,
                                                   m * P:(m + 1) * P],
                                                wvt[:, ti:ti + 2, :],
                                                start=(t == 0), stop=False,
                                                perf_mode=DR)
                                    for ci in range(2):
                                        wvt, wgvt = wvts[ci]
                                        for ti in range(0, TC, 2):
                                            t = ci * TC + ti
                                            nc.tensor.matmul(
                                                vgate[:],
                                                h1[:, t:t + 2,
                                                   m * P:(m + 1) * P],
                                                wgvt[:, ti:ti + 2, :],
                                                start=(t == 0), stop=False,
                                                perf_mode=DR)
                                    nc.tensor.matmul(
                                        vmain[:], ones_row[:],
                                        bvrow[:, n * S:(n + 1) * S],
                                        start=False, stop=True)
                                    nc.tensor.matmul(
                                        vgate[:], ones_row[:],
                                        bgvrow[:, n * S:(n + 1) * S],
                                        start=False, stop=True)
                                    vsig = wsA.tile([P, S], bf16,
                                                    tag="vsig", bufs=3,
                                                    name=f"vsig_{n}_{m}")
                                    nc.scalar.activation(vsig[:], vgate[:],
                                                         AF.Sigmoid,
                                                         scale=1.0 / WS)
                                    vout = wsA.tile([P, S], f8,
                                                    tag="vout", bufs=3,
                                                    name=f"vout_{n}_{m}")
                                    nc.vector.tensor_tensor(
                                        vout[:], vmain[:], vsig[:],
                                        op=ALU.mult)
                                    nc.scalar.dma_start(
                                        vN_bounce[m * P:(m + 1) * P,
                                                  n * S:(n + 1) * S],
                                        vout[:])

                            nc.gpsimd.collective_compute(
                                "AllGather", ALU.bypass, ins=[vN_bounce[:]],
                                outs=[vgN[:]], replica_groups=RG)

                            # V loads right behind AG_V on the gpsimd queue
                            for kb in range(NKB):
                                nc.gpsimd.dma_start(Vt[:, kb, :],
                                                    vgN_v[:, kb, :])

                            # ---- K projection + AllGather ----
                            def k_epi(j, main, sig):
                                kv = wsA.tile([P, S], f8, tag="kv_out",
                                              name=f"kv_k_{j}", bufs=3)
                                nc.vector.scalar_tensor_tensor(
                                    kv[:], main[:], bias["bk"][:, j:j + 1],
                                    sig[:], op0=ALU.add, op1=ALU.mult)
                                nc.scalar.dma_start(
                                    k_bounce[j * P:(j + 1) * P, :], kv[:])

                            proj_gated(h1, h1, NT, NT, "Wk", "Wgk", "bgk",
                                       wsA, pjpsA, k_epi, 1.0 / WS)

                            nc.gpsimd.collective_compute(
                                "AllGather", ALU.bypass, ins=[k_bounce[:]],
                                outs=[kg[:]], replica_groups=RG)

                            for hh in range(4):
                                load_Kh(hh)

                            # ---- Q projection ----
                            def q_epi(j, main, sig):
                                nc.vector.scalar_tensor_tensor(
                                    q[:, j, :], main[:],
                                    bias["bq"][:, j:j + 1],
                                    sig[:], op0=ALU.add, op1=ALU.mult)

                            proj_gated(h1, h1, NT, NT, "Wq", "Wgq", "bgq",
                                       wsA, pjpsA, q_epi, 1.0 / WS)

                    # ---- phase B: attention ----
                    with (
                        tc.tile_pool(name="apool", bufs=4) as apool,
                        tc.tile_pool(name="atps", bufs=1,
                                     space="PSUM") as atps,
                    ):
                        y = ypool.tile([P, NT, S], f8)
                        head_state = {}
                        NP = NKB // 2  # At pairs per head

                        def finalize_head(h, Zp_h, Yp_h):
                            urow = rows.tile([1, S], fp32, name=f"u_{h}",
                                             tag="urow", bufs=2)
                            nc.vector.reciprocal_approx_fast(urow[:],
                                                             Zp_h[:])
                            ubf = rows.tile([1, S], bf16, name=f"ubf_{h}",
                                            tag="ubf", bufs=2)
                            nc.vector.tensor_copy(ubf[:], urow[:])
                            Up = atps.tile([P, S], fp32, name=f"Up_{h}",
                                           tag="logits", bufs=4)
                            nc.tensor.matmul(Up[:], ones_row[:], ubf[:])
                            Us = apool.tile([P, S], bf16, tag="Us",
                                            name=f"Us_{h}")
                            nc.vector.tensor_copy(Us[:], Up[:])
                            nc.vector.tensor_tensor(y[:, h, :], Yp_h[:],
                                                    Us[:], op=ALU.mult)

                        for hh in range(H):
                            if hh not in khs:
                                load_Kh(hh)
                            Kh = khs[hh]
                            Yp = atps.tile([P, S], fp32, name=f"Y_{hh}",
                                           tag="Yp", bufs=2)
                            prs = {}
                            zsum = {}

                            def do_pair(pi, hh=hh, Kh=Kh, prs=prs):
                                Atp = apool.tile([P, 2, S], f8, tag="At",
                                                 name=f"At_{hh}_{pi}",
                                                 bufs=4)
                                for i in range(2):
                                    kb = 2 * pi + i
                                    Lp = atps.tile([P, S], fp32,
                                                   name=f"L_{hh}_{kb}",
                                                   tag="logits", bufs=4)
                                    nc.tensor.matmul(
                                        Lp[:],
                                        Kh[:, kb // GROUP,
                                           (kb % GROUP) * P:
                                           (kb % GROUP + 1) * P],
                                        q[:, hh, :])
                                    nc.scalar.activation(
                                        Atp[:, i, :], Lp[:], AF.Exp,
                                        scale=ISCALE / (WS * WS))
                                prs[pi] = Atp

                            do_pair(0)
                            for pi in range(NP):
                                if pi + 1 < NP:
                                    do_pair(pi + 1)
                                # softmax denominator on DVE (frees PE)
                                for i in range(2):
                                    if pi == 0 and i == 0:
                                        continue
                                    zs = apool.tile([P, S], fp32,
                                                    tag="Ssum", bufs=3,
                                                    name=f"Zs_{hh}_{pi}_{i}")
                                    if pi == 0 and i == 1:
                                        nc.vector.tensor_tensor(
                                            zs[:], prs[0][:, 0, :],
                                            prs[0][:, 1, :], op=ALU.add)
                                    else:
                                        nc.vector.tensor_tensor(
                                            zs[:], zsum[0][:],
                                            prs[pi][:, i, :], op=ALU.add)
                                    zsum[0] = zs
                                nc.tensor.matmul(
                                    Yp[:],
                                    Vt[:, 2 * pi:2 * pi + 2,
                                       hh * P:(hh + 1) * P],
                                    prs[pi][:],
                                    start=(pi == 0),
                                    stop=(pi == NP - 1),
                                    perf_mode=DR)
                                if pi == 2 and hh > 0:
                                    finalize_head(hh - 1,
                                                  *head_state[hh - 1])
                            Zp = atps.tile([1, S], fp32, name=f"Z_{hh}",
                                           tag="Zp", bufs=2)
                            nc.tensor.matmul(Zp[:], ones_col_f[:],
                                             zsum[0][:])
                            head_state[hh] = (Zp, Yp)
                        finalize_head(H - 1, *head_state[H - 1])

                  # ---- phase C: o-proj + residual ----
                  x2 = x2p.tile([P, NT, S], fp32, name="x2")
                  with (
                      tc.tile_pool(name="wsO", bufs=1) as wsO,
                      tc.tile_pool(name="pjpsC", bufs=1,
                                   space="PSUM") as pjpsC,
                  ):
                      def o_epi(j, main, sig):
                          xr = wsO.tile([P, S], fp32, tag="xres",
                                        name=f"xres_{j}", bufs=3)
                          DQ[j % 3].dma_start(xr[:], xT_v[j])
                          tmp = wsO.tile([P, S], fp32, tag="o_tmp",
                                         name=f"o_tmp_{j}", bufs=3)
                          nc.vector.scalar_tensor_tensor(
                              tmp[:], main[:], bias["bo"][:, j:j + 1],
                              sig[:], op0=ALU.add, op1=ALU.mult)
                          nc.vector.scalar_tensor_tensor(
                              x2[:, j, :], tmp[:], invo[:],
                              xr[:], op0=ALU.mult, op1=ALU.add)

                      proj_gated(y, y, NT, NT, "Wo", "Wgo", "bgo",
                                 wsO, pjpsC, o_epi, 1.0 / (WS * WS))

              # ---- phase D: LN2 + MLP ----
              with (
                  tc.tile_pool(name="midp", bufs=1) as midp,
                  tc.tile_pool(name="mid8p", bufs=1) as mid8p,
                  tc.tile_pool(name="pjpsD", bufs=1, space="PSUM") as pjpsD,
              ):
                  mid = midp.tile([P, NF, S], bf16)
                  mid8 = mid8p.tile([P, NF, S], f8)
                  with tc.tile_pool(name="h2p", bufs=1) as h2p:
                      with tc.tile_pool(name="wsD", bufs=1) as wsD:
                          with tc.tile_pool(name="ln2tmp", bufs=1) as ln2tmp:
                              h2 = ln_T(
                                  lambda t, p: x2[:, t, :], h2p, ln2tmp,
                                  pjpsD, "ln2", out_dt=in_main_dt)
                              if FP8_IN_MAIN:
                                  h2m = h2g = h2
                              else:
                                  h2g8 = h2p.tile([P, NT, S], f8,
                                                  name="h2g8")
                                  for t in range(NT):
                                      nc.vector.tensor_copy(h2g8[:, t, :],
                                                            h2[:, t, :])
                                  h2m, h2g = h2, h2g8

                          def mid_epi(j, main, sig):
                              tmp = wsD.tile([P, S], fp32, tag="mid_tmp",
                                             name=f"mid_tmp_{j}", bufs=3)
                              nc.vector.scalar_tensor_tensor(
                                  tmp[:], main[:],
                                  bias["bin"][:, j:j + 1], sig[:],
                                  op0=ALU.add, op1=ALU.mult)
                              nc.scalar.activation(
                                  mid[:, j, :], tmp[:], AF.Gelu,
                                  scale=(1.0 / WS if FP8_IN_MAIN else 1.0))
                              nc.vector.tensor_copy(mid8[:, j, :],
                                                    mid[:, j, :])

                          proj_gated(h2m, h2g, NT, NF, "Win", "Wgin",
                                     "bgin", wsD, pjpsD, mid_epi, 1.0 / WS,
                                     m_fp8=FP8_IN_MAIN)

                  with tc.tile_pool(name="wsE", bufs=1) as wsE:
                      def out_epi(j, main, sig):
                          tmp = wsE.tile([P, S], fp32, tag="out_tmp",
                                         name=f"out_tmp_{j}", bufs=3)
                          nc.vector.scalar_tensor_tensor(
                              tmp[:], main[:], bias["bout"][:, j:j + 1],
                              sig[:], op0=ALU.add, op1=ALU.mult)
                          outf = wsE.tile([P, S], fp32, tag="out_f",
                                          name=f"out_f_{j}", bufs=3)
                          nc.vector.tensor_tensor(outf[:], tmp[:],
                                                  x2[:, j, :], op=ALU.add)
                          DQ[j % 3].dma_start(
                              out_d.ap()[j * P:(j + 1) * P, :], outf[:])

                      proj_gated(mid, mid8, NF, NT, "Wout", "Wgout",
                                 "bgout", wsE, pjpsD, out_epi, 1.0 / WS,
                                 m_fp8=False, tchunk=32, wbufs=2)

    nc.compile()
    return nc


def _prep_shared_inputs(inputs):
    f32 = np.float32
    g1 = np.asarray(inputs["ln1_g"], f32)
    b1 = np.asarray(inputs["ln1_b"], f32)
    g2 = np.asarray(inputs["ln2_g"], f32)
    b2 = np.asarray(inputs["ln2_b"], f32)

    # Fold the LN affine into the consumer weights/biases:
    #   h_affine = h_norm * g + b  =>  W' = W*g[None,:], b' = b_proj + W@b
    def fold(wn, bn, g, bln):
        W = np.asarray(inputs[wn], f32)
        bb = np.asarray(inputs[bn], f32)
        return W * g[None, :], bb + W @ bln

    m = {}
    folded_b = {}
    for nm, wn, bn, g, bln in (
            ("Wq", "W_q", "b_q", g1, b1), ("Wgq", "Wg_q", "bg_q", g1, b1),
            ("Wk", "W_k", "b_k", g1, b1), ("Wgk", "Wg_k", "bg_k", g1, b1),
            ("Win", "W_in", "b_in", g2, b2),
            ("Wgin", "Wg_in", "bg_in", g2, b2)):
        W, bb = fold(wn, bn, g, bln)
        if nm == "Win" and not FP8_IN_MAIN:
            m[nm] = _w_tiled_bf(W)
        else:
            m[nm] = _w_tiled_f8(W)
        folded_b[bn] = bb
    for nm, wn in (("Wo", "W_o"), ("Wgo", "Wg_o"), ("Wgout", "Wg_out")):
        m[nm] = _w_tiled_f8(np.asarray(inputs[wn], f32))
    m["Wout"] = _w_tiled_bf(np.asarray(inputs["W_out"], f32))
    Wv, bv = fold("W_v", "b_v", g1, b1)
    Wgv, bgv = fold("Wg_v", "bg_v", g1, b1)
    m["WvT"] = np.ascontiguousarray(
        np.clip(Wv.T * WS, -240, 240).astype(_F8))
    m["WgvT"] = np.ascontiguousarray(
        np.clip(Wgv.T * WS, -240, 240).astype(_F8))
    m["bvrow"] = (bv * WS).astype(_BF).reshape(1, D)
    m["bgvrow"] = (bgv * WS).astype(_BF).reshape(1, D)
    main_b_scale = {"bq": WS, "bk": WS, "bo": WS * WS,
                    "bin": (WS if FP8_IN_MAIN else 1.0), "bout": 1.0}
    for nm, bn in (("bq", "b_q"), ("bgq", "bg_q"), ("bk", "b_k"),
                   ("bgk", "bg_k"), ("bo", "b_o"), ("bgo", "bg_o"),
                   ("bin", "b_in"), ("bgin", "bg_in"), ("bout", "b_out"),
                   ("bgout", "bg_out")):
        bb = folded_b.get(bn, None)
        if bb is None:
            bb = np.asarray(inputs[bn], f32)
        m[nm] = _b_cols(bb, main_b_scale.get(nm, 1.0))
    return m


def _install_trace_shim():
    """Provide antenv.axon_hooks (NTFF profiling) if the image lacks it."""
    import contextlib
    import ctypes
    import types

    try:
        import antenv.axon_hooks  # noqa: F401
        return
    except ImportError:
        pass
    try:
        import antenv
    except ImportError:
        return
    so_path = "/opt/axon/libaxon_pjrt.so"
    try:
        lib = ctypes.CDLL(so_path)
    except OSError:
        return
    if not hasattr(lib, "axon_start_nrt_profile"):
        return
    lib.axon_start_nrt_profile.argtypes = [ctypes.POINTER(ctypes.c_int64),
                                           ctypes.c_size_t]
    lib.axon_start_nrt_profile.restype = ctypes.c_int64
    lib.axon_stop_nrt_profile.argtypes = [ctypes.c_char_p]
    lib.axon_stop_nrt_profile.restype = ctypes.c_int64

    @contextlib.contextmanager
    def hook(output_dir, device_ids):
        import jax

        jax.devices()
        if device_ids:
            ids = (ctypes.c_int64 * len(device_ids))(*device_ids)
            rc = lib.axon_start_nrt_profile(ids, len(device_ids))
        else:
            rc = lib.axon_start_nrt_profile(None, 0)
        if rc != 0:
            raise RuntimeError(f"axon_start_nrt_profile rc={rc}")
        try:
            yield
        finally:
            n = lib.axon_stop_nrt_profile(str(output_dir).encode())
            print(f"profile: {n} ntff file(s) in {output_dir}",
                  file=sys.stderr)

    mod = types.ModuleType("antenv.axon_hooks")
    mod.get_axon_ntff_profile_hook = lambda: hook
    mod.set_axon_ntff_profile_hook = lambda h: None
    sys.modules["antenv.axon_hooks"] = mod
    antenv.axon_hooks = mod


LAST_RESULTS = None


def kernel(_trace=False, **inputs):
    global _COMPILED, LAST_RESULTS
    from concourse import bass_utils

    if _trace:
        _install_trace_shim()

    if _COMPILED is None:
        _COMPILED = _build()
    nc = _COMPILED

    shared = _prep_shared_inputs(inputs)
    x = np.asarray(inputs["x"], dtype=np.float32)  # [B, T, D]
    in_maps = []
    for c in range(N_CORES):
        g, s = divmod(c, GROUP)
        xT_c = np.ascontiguousarray(x[g, s * S:(s + 1) * S, :].T)
        m = dict(shared)
        m["xT"] = xT_c
        in_maps.append(m)

    LAST_RESULTS = bass_utils.run_bass_kernel_spmd(
        nc, in_maps, core_ids=list(range(N_CORES)), trace=_trace)

    out = np.empty((B, T, D), dtype=np.float32)
    for c in range(N_CORES):
        g, s = divmod(c, GROUP)
        out[g, s * S:(s + 1) * S, :] = LAST_RESULTS.results[c]["outT"].T
    return out


# revision 19
# speedup vs baseline: 1.0585x; 1.0080x over previous
"""Trainium2 Bass kernel for nn_DGEBlock (dense transformer block with
MoE-gated linears), distributed over 8 NeuronCores.

Sharding: data-parallel over batch (2 groups of 4 cores) x sequence-parallel
over tokens within each batch (512 tokens per core). Weights are replicated
(host pre-tiled); activations live feature-major ("T-layout": [d, tok]) in
SBUF so projections are lhsT=W^T-tile @ rhs=activation with no activation
transposes. V is projected in token-major (N-)layout directly so attention's
PV matmuls need no transposes either.

FP8 (e4m3) with DoubleRow double-pumping on the TensorEngine for the
projection matmuls: q/k/v/o main+gate, mlp_in main+gate, mlp_out gate.
Weights are pre-scaled x16 on the host so their values sit in the fp8
normal range; descaling is deferred into existing epilogue scalars. The
LN affine (gain/shift) is folded into the downstream weights host-side.
mlp_out's main matmul stays bf16 (its quantization error dominates the
output). Attention: QK in plain fp8, exp/PV in bf16 (exp range exceeds
fp8), softmax denominator accumulated on the Vector engine.

Scheduling: weight-stream and PSUM pools are opened BEFORE the LN tmp
pools so weight DMA issue never waits for LN to finish; collectives are
gathered fp8 and sequenced on the gpsimd queue as [AG_V, Vt8-loads,
AG_K, Kh-preloads] so both hide under the K/Q projections; per-tile
weight loads are single rearranged-AP DMA descriptors (the sequencer
pays ~600ns per issue); x/out tiles round-robin across the three
DMA-capable queues.
"""

import sys

for _p in ("/opt/trn_rl_repo",):
    if _p not in sys.path:
        sys.path.append(_p)

import numpy as np
import ml_dtypes

# ---------------------------------------------------------------- constants
B = 2
T = 2048
D = 2048
H = 16
HD = 128
FF = 4 * D  # 8192
EPS = 1e-5

N_CORES = 8
GROUP = 4  # cores per batch group (sequence-parallel degree)
S = T // GROUP  # tokens per core = 512
P = 128
NT = D // P  # 16 feature tiles
NF = FF // P  # 64 hidden tiles
NKB = T // P  # 16 key blocks per batch
ISCALE = 1.0 / float(np.sqrt(HD))

WS = 16.0  # fp8 weight pre-scale
FP8_IN_MAIN = True  # mlp_in main matmul in fp8 (False -> bf16 fallback)

RG = [[0, 1, 2, 3], [4, 5, 6, 7]]

_BF = ml_dtypes.bfloat16
_F8 = ml_dtypes.float8_e4m3

_COMPILED = None


# ------------------------------------------------------------- host prep
def _tile4(W):
    """W [dout, din] -> [nj, 128, nt, 128] such that
    out[j, p, t, jc] == W[j*128+jc, t*128+p]  (= W^T tile (t, j))."""
    dout, din = W.shape
    nj, nt = dout // P, din // P
    return W.reshape(nj, P, nt, P).transpose(0, 3, 2, 1)


def _w_tiled_bf(W):
    return np.ascontiguousarray(_tile4(W).astype(_BF))


def _w_tiled_f8(W):
    return np.ascontiguousarray(
        np.clip(_tile4(W) * WS, -240, 240).astype(_F8)
    )


def _b_cols(b, scale=1.0):
    """b [dout] -> [128, nj] fp32: column j holds b[j*128:(j+1)*128]."""
    nj = b.shape[0] // P
    return np.ascontiguousarray((b * scale).reshape(nj, P).T.astype(np.float32))


# ------------------------------------------------------------- device build
def _build():
    from concourse import bacc, tile, mybir

    fp32 = mybir.dt.float32
    bf16 = mybir.dt.bfloat16
    f8 = mybir.dt.float8e4
    AF = mybir.ActivationFunctionType
    ALU = mybir.AluOpType
    DR = mybir.MatmulPerfMode.DoubleRow

    in_main_dt = f8 if FP8_IN_MAIN else bf16

    nc = bacc.Bacc("TRN2", target_bir_lowering=False, debug=False,
                   num_devices=N_CORES)

    # ---- I/O tensors
    xT_d = nc.dram_tensor("xT", [D, S], fp32, kind="ExternalInput")
    wd = {}
    for nm in ("Wq", "Wgq", "Wk", "Wgk", "Wo", "Wgo"):
        wd[nm] = nc.dram_tensor(nm, [NT, P, NT, P], f8, kind="ExternalInput")
    wd["Win"] = nc.dram_tensor("Win", [NF, P, NT, P], in_main_dt,
                               kind="ExternalInput")
    wd["Wgin"] = nc.dram_tensor("Wgin", [NF, P, NT, P], f8,
                                kind="ExternalInput")
    wd["Wout"] = nc.dram_tensor("Wout", [NT, P, NF, P], bf16,
                                kind="ExternalInput")
    wd["Wgout"] = nc.dram_tensor("Wgout", [NT, P, NF, P], f8,
                                 kind="ExternalInput")
    # V projection runs in N-layout: plain W^T [din, dout] + bias rows
    wd["WvT"] = nc.dram_tensor("WvT", [D, D], f8, kind="ExternalInput")
    wd["WgvT"] = nc.dram_tensor("WgvT", [D, D], f8, kind="ExternalInput")
    bvrow_d = nc.dram_tensor("bvrow", [1, D], bf16, kind="ExternalInput")
    bgvrow_d = nc.dram_tensor("bgvrow", [1, D], bf16, kind="ExternalInput")
    bd = {}
    for nm in ("bq", "bgq", "bk", "bgk", "bo", "bgo", "bout", "bgout"):
        bd[nm] = nc.dram_tensor(nm, [P, NT], fp32, kind="ExternalInput")
    for nm in ("bin", "bgin"):
        bd[nm] = nc.dram_tensor(nm, [P, NF], fp32, kind="ExternalInput")
    out_d = nc.dram_tensor("outT", [D, S], fp32, kind="ExternalOutput")

    with tile.TileContext(nc) as tc:
        with (
            tc.tile_pool(name="const", bufs=1) as constp,
            tc.tile_pool(name="bias", bufs=1) as biasp,
            tc.tile_pool(name="rows", bufs=1) as rows,
            tc.tile_pool(name="dram", bufs=1, space="DRAM") as dramp,
        ):
            DQ = [nc.sync, nc.gpsimd, nc.scalar]

            ones_col = constp.tile([P, 1], bf16)
            nc.vector.memset(ones_col[:], 1.0)
            ones_col_f = constp.tile([P, 1], fp32)
            nc.vector.memset(ones_col_f[:], 1.0)
            ones_row = constp.tile([1, P], bf16)
            nc.vector.memset(ones_row[:], 1.0)
            eps_t = constp.tile([1, 1], fp32)
            nc.vector.memset(eps_t[:], EPS)
            invo = constp.tile([P, 1], fp32)
            nc.vector.memset(invo[:], 1.0 / (WS * WS))

            bias = {}

            def load_consts():
                bvrow = constp.tile([1, D], bf16)
                nc.scalar.dma_start(bvrow[:], bvrow_d.ap())
                bgvrow = constp.tile([1, D], bf16)
                nc.scalar.dma_start(bgvrow[:], bgvrow_d.ap())
                for nm in bd:
                    ncols = NF if nm in ("bin", "bgin") else NT
                    btile = biasp.tile([P, ncols], fp32, name=f"bias_{nm}")
                    nc.scalar.dma_start(btile[:], bd[nm].ap())
                    bias[nm] = btile
                return bvrow, bgvrow

            # ---------- helpers ----------
            def hsl(hs, t, w, cols=None):
                g, o = divmod(t, 4)
                if cols is None:
                    return hs[g][:, o:o + w, :]
                return hs[g][:, o:o + w, cols[0]:cols[1]]

            def ln_T(get_src, hpool, tmpool, psln, name, out_dt=f8):
                """LayerNorm (stats+normalize only; affine folded into
                consumer weights host-side). get_src(t, pass_idx) yields
                [128, S] fp32 tiles -> out_dt [128, NT, S]. The per-tile
                normalize runs the multiply on GpSimd and the subtract
                (+fp8 cast) on DVE."""
                S1 = psln.tile([1, S], fp32, name=f"{name}_S1",
                               tag="lnS", bufs=2)
                S2 = psln.tile([1, S], fp32, name=f"{name}_S2",
                               tag="lnS", bufs=2)
                for t in range(NT):
                    srct = get_src(t, 0)
                    xbf = tmpool.tile([P, S], bf16, name=f"{name}_xbf_{t}",
                                      tag="ln_xbf", bufs=2)
                    nc.vector.tensor_copy(xbf[:], srct)
                    sq = tmpool.tile([P, S], bf16, name=f"{name}_sq_{t}",
                                     tag="ln_sq", bufs=2)
                    nc.scalar.activation(sq[:], srct, AF.Square)
                    nc.tensor.matmul(S1[:], ones_col[:], xbf[:],
                                     start=(t == 0), stop=(t == NT - 1))
                    nc.tensor.matmul(S2[:], ones_col[:], sq[:],
                                     start=(t == 0), stop=(t == NT - 1))

                def row(nm, dt=fp32):
                    return rows.tile([1, S], dt, name=f"{name}_{nm}",
                                     tag=f"ln_{nm}")

                mean = row("mean")
                nc.vector.tensor_scalar_mul(mean[:], S1[:], 1.0 / D)
                m2 = row("m2")
                nc.vector.tensor_scalar_mul(m2[:], S2[:], 1.0 / D)
                msq = row("msq")
                nc.vector.tensor_tensor(msq[:], mean[:], mean[:],
                                        op=ALU.mult)
                var = row("var")
                nc.vector.tensor_tensor(var[:], m2[:], msq[:],
                                        op=ALU.subtract)
                std = row("std")
                nc.scalar.activation(std[:], var[:], AF.Sqrt, bias=eps_t[:])
                rstd = row("rstd")
                nc.vector.reciprocal_approx_fast(rstd[:], std[:])
                rstd_bf = row("rstdbf", bf16)
                nc.vector.tensor_copy(rstd_bf[:], rstd[:])
                mr_bf = row("mrbf", bf16)
                nc.vector.tensor_tensor(mr_bf[:], mean[:], rstd[:],
                                        op=ALU.mult)
                Ab_p = psln.tile([P, S], fp32, name=f"{name}_Abp",
                                 tag="pj_main", bufs=3)
                nc.tensor.matmul(Ab_p[:], ones_row[:], rstd_bf[:])
                Bb_p = psln.tile([P, S], fp32, name=f"{name}_Bbp",
                                 tag="pj_gate", bufs=3)
                nc.tensor.matmul(Bb_p[:], ones_row[:], mr_bf[:])
                Ab = tmpool.tile([P, S], fp32, name=f"{name}_Ab")
                nc.vector.tensor_copy(Ab[:], Ab_p[:])
                Bb = tmpool.tile([P, S], fp32, name=f"{name}_Bb")
                nc.vector.tensor_copy(Bb[:], Bb_p[:])
                hs = [hpool.tile([P, 4, S], out_dt,
                                 name=f"{name}_h{g}") for g in range(4)]
                for t in range(NT):
                    srct = get_src(t, 1)
                    tmp = tmpool.tile([P, S], fp32, name=f"{name}_t0_{t}",
                                      tag="ln_t0", bufs=3)
                    nc.gpsimd.tensor_tensor(tmp[:], srct, Ab[:],
                                            op=ALU.mult)
                    nc.vector.tensor_tensor(hs[t // 4][:, t % 4, :],
                                            tmp[:], Bb[:],
                                            op=ALU.subtract)
                return hs

            def accum(psum, src, wname, j, nt, tchunk, wpool, wbufs, fp8,
                      tag):
                """psum += sum_t W^T(t,j).T @ src[:,t,:] (DoubleRow pairs
                when fp8). One DMA descriptor per weight tile."""
                nchunk = nt // tchunk
                wdt = f8 if fp8 else bf16
                for ci in range(nchunk):
                    wt = wpool.tile([P, tchunk, P], wdt, tag=tag,
                                    name=f"w_{wname}_{j}_{ci}", bufs=wbufs)
                    nc.sync.dma_start(
                        wt[:],
                        wd[wname].ap()[j, :,
                                       ci * tchunk:(ci + 1) * tchunk, :])
                    if fp8:
                        for ti in range(0, tchunk, 2):
                            t = ci * tchunk + ti
                            rhs = (hsl(src, t, 2) if isinstance(src, list)
                                   else src[:, t:t + 2, :])
                            nc.tensor.matmul(psum[:], wt[:, ti:ti + 2, :],
                                             rhs,
                                             start=(t == 0),
                                             stop=(t == nt - 2),
                                             perf_mode=DR)
                    else:
                        for ti in range(tchunk):
                            t = ci * tchunk + ti
                            rhs = (hsl(src, t, 1) if isinstance(src, list)
                                   else src[:, t, :])
                            nc.tensor.matmul(psum[:], wt[:, ti, :],
                                             rhs,
                                             start=(t == 0),
                                             stop=(t == nt - 1))

            def proj_gated(src_m, src_g, nt, nj, wname, wgname, bgname,
                           wpool, pspool, epilogue, gsc, m_fp8=True,
                           tchunk=None, wbufs=3):
                """Gated projection in T-layout. Gate path is always fp8;
                main path fp8 iff m_fp8."""
                if tchunk is None:
                    tchunk = nt
                for j in range(nj):
                    main = pspool.tile([P, S], fp32, name=f"{wname}_m{j}",
                                       tag="pj_main", bufs=3)
                    gate = pspool.tile([P, S], fp32, name=f"{wname}_g{j}",
                                       tag="pj_gate", bufs=3)
                    accum(main, src_m, wname, j, nt, tchunk, wpool, wbufs,
                          m_fp8, "wmain")
                    accum(gate, src_g, wgname, j, nt, tchunk, wpool, wbufs,
                          True, "wgate")
                    sig = wpool.tile([P, S], bf16, tag="sig",
                                     name=f"sig_{wname}_{j}", bufs=3)
                    nc.scalar.activation(sig[:], gate[:], AF.Sigmoid,
                                         bias=bias[bgname][:, j:j + 1],
                                         scale=gsc)
                    epilogue(j, main, sig)

            with tc.tile_pool(name="x2p", bufs=1) as x2p:
              with tc.tile_pool(name="xlnp", bufs=1) as xlnp:
                xT_v = xT_d.ap().rearrange("(t p) s -> t p s", p=P)

                def x_src(t, pass_idx):
                    xa = xlnp.tile([P, S], fp32, tag=f"xln{pass_idx}",
                                   bufs=3,
                                   name=f"x_{pass_idx}_{t}")
                    DQ[t % 3].dma_start(xa[:], xT_v[t])
                    return xa[:]

                bvrow, bgvrow = load_consts()

                vN_bounce = dramp.tile([S, D], f8)
                k_bounce = dramp.tile([D, S], f8)
                vgN = dramp.tile([GROUP * S, D], f8)
                kg = dramp.tile([GROUP * D, S], f8)
                kg_v = kg[:, :].rearrange("(s d) c -> d s c", d=D)
                vgN_v = vgN[:, :].rearrange("(kb p) c -> p kb c", p=P)
                WvT_v = wd["WvT"].ap().rearrange("(t p) c -> p t c", p=P)
                WgvT_v = wd["WgvT"].ap().rearrange("(t p) c -> p t c", p=P)

                with tc.tile_pool(name="yp", bufs=1) as ypool:
                  with tc.tile_pool(name="qp", bufs=1) as qpool, \
                       tc.tile_pool(name="kstream", bufs=1) as kpool, \
                       tc.tile_pool(name="vres", bufs=1) as vresp:
                    q = qpool.tile([P, NT, S], f8)
                    Vt = vresp.tile([P, NKB, D], f8)
                    khs = {}

                    def load_Kh(hh):
                        Kh = kpool.tile([P, GROUP, S], f8, tag="Kh",
                                        name=f"Kh_{hh}", bufs=5)
                        nc.gpsimd.dma_start(Kh[:],
                                            kg_v[hh * P:(hh + 1) * P])
                        khs[hh] = Kh

                    with (
                        tc.tile_pool(name="wsA", bufs=1) as wsA,
                        tc.tile_pool(name="pjpsA", bufs=1,
                                     space="PSUM") as pjpsA,
                    ):
                        with tc.tile_pool(name="hq", bufs=1) as hqp:
                            with tc.tile_pool(name="ln1tmp",
                                              bufs=1) as ln1tmp:
                                h1 = ln_T(x_src, hqp, ln1tmp, pjpsA,
                                          "ln1")

                            # ---- V projection (N-layout, m-outer) ----
                            TC = NT // 2
                            for n in range(4):
                                wvts = []
                                for ci in range(2):
                                    wvt = wsA.tile([P, TC, 4 * P], f8,
                                                   tag="wv", bufs=3,
                                                   name=f"wv_{n}_{ci}")
                                    nc.sync.dma_start(
                                        wvt[:],
                                        WvT_v[:, ci * TC:(ci + 1) * TC,
                                              n * S:(n + 1) * S])
                                    wgvt = wsA.tile([P, TC, 4 * P], f8,
                                                    tag="wgv", bufs=3,
                                                    name=f"wgv_{n}_{ci}")
                                    nc.sync.dma_start(
                                        wgvt[:],
                                        WgvT_v[:, ci * TC:(ci + 1) * TC,
                                               n * S:(n + 1) * S])
                                    wvts.append((wvt, wgvt))
                                for m in range(4):
                                    vmain = pjpsA.tile([P, S], fp32,
                                                       tag="pj_main",
                                                       bufs=3,
                                                       name=f"vm_{n}_{m}")
                                    vgate = pjpsA.tile([P, S], fp32,
                                                       tag="pj_gate",
                                                       bufs=3,
                                                       name=f"vg_{n}_{m}")
                                    for ci in range(2):
                                        wvt, wgvt = wvts[ci]
                                        for ti in range(0, TC, 2):
                                            t = ci * TC + ti
                                            nc.tensor.matmul(
                                                vmain[:],
                                                hsl(h1, t, 2,
                                                    (m * P, (m + 1) * P)),
                                                wvt[:, ti:ti + 2, :],
                                                start=(t == 0), stop=False,
                                                perf_mode=DR)
                                    for ci in range(2):
                                        wvt, wgvt = wvts[ci]
                                        for ti in range(0, TC, 2):
                                            t = ci * TC + ti
                                            nc.tensor.matmul(
                                                vgate[:],
                                                hsl(h1, t, 2,
                                                    (m * P, (m + 1) * P)),
                                                wgvt[:, ti:ti + 2, :],
                                                start=(t == 0), stop=False,
                                                perf_mode=DR)
                                    nc.tensor.matmul(
                                        vmain[:], ones_row[:],
                                        bvrow[:, n * S:(n + 1) * S],
                                        start=False, stop=True)
                                    nc.tensor.matmul(
                                        vgate[:], ones_row[:],
                                        bgvrow[:, n * S:(n + 1) * S],
                                        start=False, stop=True)
                                    vsig = wsA.tile([P, S], bf16,
                                                    tag="vsig", bufs=3,
                                                    name=f"vsig_{n}_{m}")
                                    nc.scalar.activation(vsig[:], vgate[:],
                                                         AF.Sigmoid,
                                                         scale=1.0 / WS)
                                    vout = wsA.tile([P, S], f8,
                                                    tag="vout", bufs=3,
                                                    name=f"vout_{n}_{m}")
                                    nc.vector.tensor_tensor(
                                        vout[:], vmain[:], vsig[:],
                                        op=ALU.mult)
                                    nc.scalar.dma_start(
                                        vN_bounce[m * P:(m + 1) * P,
                                                  n * S:(n + 1) * S],
                                        vout[:])

                            nc.gpsimd.collective_compute(
                                "AllGather", ALU.bypass, ins=[vN_bounce[:]],
                                outs=[vgN[:]], replica_groups=RG)

                            # V loads right behind AG_V on the gpsimd queue
                            for kb in range(NKB):
                                nc.gpsimd.dma_start(Vt[:, kb, :],
                                                    vgN_v[:, kb, :])

                            # ---- K projection + AllGather ----
                            def k_epi(j, main, sig):
                                kv = wsA.tile([P, S], f8, tag="kv_out",
                                              name=f"kv_k_{j}", bufs=3)
                                nc.vector.scalar_tensor_tensor(
                                    kv[:], main[:], bias["bk"][:, j:j + 1],
                                    sig[:], op0=ALU.add, op1=ALU.mult)
                                nc.scalar.dma_start(
                                    k_bounce[j * P:(j + 1) * P, :], kv[:])

                            proj_gated(h1, h1, NT, NT, "Wk", "Wgk", "bgk",
                                       wsA, pjpsA, k_epi, 1.0 / WS)

                            nc.gpsimd.collective_compute(
                                "AllGather", ALU.bypass, ins=[k_bounce[:]],
                                outs=[kg[:]], replica_groups=RG)

                            for hh in range(4):
                                load_Kh(hh)

                            # ---- Q projection ----
                            def q_epi(j, main, sig):
                                nc.vector.scalar_tensor_tensor(
                                    q[:, j, :], main[:],
                                    bias["bq"][:, j:j + 1],
                                    sig[:], op0=ALU.add, op1=ALU.mult)

                            proj_gated(h1, h1, NT, NT, "Wq", "Wgq", "bgq",
                                       wsA, pjpsA, q_epi, 1.0 / WS)

                    # ---- phase B: attention ----
                    with (
                        tc.tile_pool(name="apool", bufs=4) as apool,
                        tc.tile_pool(name="atps", bufs=1,
                                     space="PSUM") as atps,
                    ):
                        y = ypool.tile([P, NT, S], f8)
                        head_state = {}
                        NP = NKB // 2  # At pairs per head

                        def finalize_head(h, Zp_h, Yp_h):
                            urow = rows.tile([1, S], fp32, name=f"u_{h}",
                                             tag="urow", bufs=2)
                            nc.vector.reciprocal_approx_fast(urow[:],
                                                             Zp_h[:])
                            ubf = rows.tile([1, S], bf16, name=f"ubf_{h}",
                                            tag="ubf", bufs=2)
                            nc.vector.tensor_copy(ubf[:], urow[:])
                            Up = atps.tile([P, S], fp32, name=f"Up_{h}",
                                           tag="logits", bufs=4)
                            nc.tensor.matmul(Up[:], ones_row[:], ubf[:])
                            Us = apool.tile([P, S], bf16, tag="Us",
                                            name=f"Us_{h}")
                            nc.vector.tensor_copy(Us[:], Up[:])
                            nc.vector.tensor_tensor(y[:, h, :], Yp_h[:],
                                                    Us[:], op=ALU.mult)

                        for hh in range(H):
                            if hh not in khs:
                                load_Kh(hh)
                            Kh = khs[hh]
                            Yp = atps.tile([P, S], fp32, name=f"Y_{hh}",
                                           tag="Yp", bufs=2)
                            prs = {}
                            zsum = {}

                            def do_pair(pi, hh=hh, Kh=Kh, prs=prs):
                                Atp = apool.tile([P, 2, S], f8, tag="At",
                                                 name=f"At_{hh}_{pi}",
                                                 bufs=4)
                                for i in range(2):
                                    kb = 2 * pi + i
                                    Lp = atps.tile([P, S], fp32,
                                                   name=f"L_{hh}_{kb}",
                                                   tag="logits", bufs=4)
                                    nc.tensor.matmul(
                                        Lp[:],
                                        Kh[:, kb // GROUP,
                                           (kb % GROUP) * P:
                                           (kb % GROUP + 1) * P],
                                        q[:, hh, :])
                                    nc.scalar.activation(
                                        Atp[:, i, :], Lp[:], AF.Exp,
                                        scale=ISCALE / (WS * WS))
                                prs[pi] = Atp

                            do_pair(0)
                            for pi in range(NP):
                                if pi + 1 < NP:
                                    do_pair(pi + 1)
                                # softmax denominator on DVE (frees PE)
                                for i in range(2):
                                    if pi == 0 and i == 0:
                                        continue
                                    zs = apool.tile([P, S], fp32,
                                                    tag="Ssum", bufs=3,
                                                    name=f"Zs_{hh}_{pi}_{i}")
                                    if pi == 0 and i == 1:
                                        nc.vector.tensor_tensor(
                                            zs[:], prs[0][:, 0, :],
                                            prs[0][:, 1, :], op=ALU.add)
                                    else:
                                        nc.vector.tensor_tensor(
                                            zs[:], zsum[0][:],
                                            prs[pi][:, i, :], op=ALU.add)
                                    zsum[0] = zs
                                nc.tensor.matmul(
                                    Yp[:],
                                    Vt[:, 2 * pi:2 * pi + 2,
                                       hh * P:(hh + 1) * P],
                                    prs[pi][:],
                                    start=(pi == 0),
                                    stop=(pi == NP - 1),
                                    perf_mode=DR)
                                if pi == 2 and hh > 0:
                                    finalize_head(hh - 1,
                                                  *head_state[hh - 1])
                            Zp = atps.tile([1, S], fp32, name=f"Z_{hh}",
                                           tag="Zp", bufs=2)
                            nc.tensor.matmul(Zp[:], ones_col_f[:],
                                             zsum[0][:])
                            head_state[hh] = (Zp, Yp)
                        finalize_head(H - 1, *head_state[H - 1])

                  # ---- phase C: o-proj + residual ----
                  x2 = x2p.tile([P, NT, S], fp32, name="x2")
                  with (
                      tc.tile_pool(name="wsO", bufs=1) as wsO,
                      tc.tile_pool(name="pjpsC", bufs=1,
                                   space="PSUM") as pjpsC,
                  ):
                      def o_epi(j, main, sig):
                          xr = wsO.tile([P, S], fp32, tag="xres",
                                        name=f"xres_{j}", bufs=3)
                          DQ[j % 3].dma_start(xr[:], xT_v[j])
                          tmp = wsO.tile([P, S], fp32, tag="o_tmp",
                                         name=f"o_tmp_{j}", bufs=3)
                          nc.vector.scalar_tensor_tensor(
                              tmp[:], main[:], bias["bo"][:, j:j + 1],
                              sig[:], op0=ALU.add, op1=ALU.mult)
                          nc.vector.scalar_tensor_tensor(
                              x2[:, j, :], tmp[:], invo[:],
                              xr[:], op0=ALU.mult, op1=ALU.add)

                      proj_gated(y, y, NT, NT, "Wo", "Wgo", "bgo",
                                 wsO, pjpsC, o_epi, 1.0 / (WS * WS))

              # ---- phase D: LN2 + MLP ----
              with (
                  tc.tile_pool(name="midp", bufs=1) as midp,
                  tc.tile_pool(name="mid8p", bufs=1) as mid8p,
                  tc.tile_pool(name="pjpsD", bufs=1, space="PSUM") as pjpsD,
              ):
                  mid = midp.tile([P, NF, S], bf16)
                  mid8 = mid8p.tile([P, NF, S], f8)
                  with tc.tile_pool(name="h2p", bufs=1) as h2p:
                      with tc.tile_pool(name="wsD", bufs=1) as wsD:
                          with tc.tile_pool(name="ln2tmp", bufs=1) as ln2tmp:
                              h2 = ln_T(
                                  lambda t, p: x2[:, t, :], h2p, ln2tmp,
                                  pjpsD, "ln2", out_dt=in_main_dt)
                              if FP8_IN_MAIN:
                                  h2m = h2g = h2
                              else:
                                  h2g8 = h2p.tile([P, NT, S], f8,
                                                  name="h2g8")
                                  for t in range(NT):
                                      nc.vector.tensor_copy(h2g8[:, t, :],
                                                            h2[:, t, :])
                                  h2m, h2g = h2, h2g8

                          def mid_epi(j, main, sig):
                              tmp = wsD.tile([P, S], fp32, tag="mid_tmp",
                                             name=f"mid_tmp_{j}", bufs=3)
                              nc.vector.scalar_tensor_tensor(
                                  tmp[:], main[:],
                                  bias["bin"][:, j:j + 1], sig[:],
                                  op0=ALU.add, op1=ALU.mult)
                              nc.scalar.activation(
                                  mid[:, j, :], tmp[:], AF.Gelu,
                                  scale=(1.0 / WS if FP8_IN_MAIN else 1.0))
                              nc.vector.tensor_copy(mid8[:, j, :],
                                                    mid[:, j, :])

                          proj_gated(h2m, h2g, NT, NF, "Win", "Wgin",
                                     "bgin", wsD, pjpsD, mid_epi, 1.0 / WS,
                                     m_fp8=FP8_IN_MAIN)

                  with tc.tile_pool(name="wsE", bufs=1) as wsE:
                      def out_epi(j, main, sig):
                          tmp = wsE.tile([P, S], fp32, tag="out_tmp",
                                         name=f"out_tmp_{j}", bufs=3)
                          nc.vector.scalar_tensor_tensor(
                              tmp[:], main[:], bias["bout"][:, j:j + 1],
                              sig[:], op0=ALU.add, op1=ALU.mult)
                          outf = wsE.tile([P, S], fp32, tag="out_f",
                                          name=f"out_f_{j}", bufs=2)
                          nc.vector.tensor_tensor(outf[:], tmp[:],
                                                  x2[:, j, :], op=ALU.add)
                          DQ[j % 3].dma_start(
                              out_d.ap()[j * P:(j + 1) * P, :], outf[:])

                      proj_gated(mid, mid8, NF, NT, "Wout", "Wgout",
                                 "bgout", wsE, pjpsD, out_epi, 1.0 / WS,
                                 m_fp8=False, tchunk=32, wbufs=3)

    nc.compile()
    return nc


def _prep_shared_inputs(inputs):
    f32 = np.float32
    g1 = np.asarray(inputs["ln1_g"], f32)
    b1 = np.asarray(inputs["ln1_b"], f32)
    g2 = np.asarray(inputs["ln2_g"], f32)
    b2 = np.asarray(inputs["ln2_b"], f32)

    # Fold the LN affine into the consumer weights/biases:
    #   h_affine = h_norm * g + b  =>  W' = W*g[None,:], b' = b_proj + W@b
    def fold(wn, bn, g, bln):
        W = np.asarray(inputs[wn], f32)
        bb = np.asarray(inputs[bn], f32)
        return W * g[None, :], bb + W @ bln

    m = {}
    folded_b = {}
    for nm, wn, bn, g, bln in (
            ("Wq", "W_q", "b_q", g1, b1), ("Wgq", "Wg_q", "bg_q", g1, b1),
            ("Wk", "W_k", "b_k", g1, b1), ("Wgk", "Wg_k", "bg_k", g1, b1),
            ("Win", "W_in", "b_in", g2, b2),
            ("Wgin", "Wg_in", "bg_in", g2, b2)):
        W, bb = fold(wn, bn, g, bln)
        if nm == "Win" and not FP8_IN_MAIN:
            m[nm] = _w_tiled_bf(W)
        else:
            m[nm] = _w_tiled_f8(W)
        folded_b[bn] = bb
    for nm, wn in (("Wo", "W_o"), ("Wgo", "Wg_o"), ("Wgout", "Wg_out")):
        m[nm] = _w_tiled_f8(np.asarray(inputs[wn], f32))
    m["Wout"] = _w_tiled_bf(np.asarray(inputs["W_out"], f32))
    Wv, bv = fold("W_v", "b_v", g1, b1)
    Wgv, bgv = fold("Wg_v", "bg_v", g1, b1)
    m["WvT"] = np.ascontiguousarray(
        np.clip(Wv.T * WS, -240, 240).astype(_F8))
    m["WgvT"] = np.ascontiguousarray(
        np.clip(Wgv.T * WS, -240, 240).astype(_F8))
    m["bvrow"] = (bv * WS).astype(_BF).reshape(1, D)
    m["bgvrow"] = (bgv * WS).astype(_BF).reshape(1, D)
    main_b_scale = {"bq": WS, "bk": WS, "bo": WS * WS,
                    "bin": (WS if FP8_IN_MAIN else 1.0), "bout": 1.0}
    for nm, bn in (("bq", "b_q"), ("bgq", "bg_q"), ("bk", "b_k"),
                   ("bgk", "bg_k"), ("bo", "b_o"), ("bgo", "bg_o"),
                   ("bin", "b_in"), ("bgin", "bg_in"), ("bout", "b_out"),
                   ("bgout", "bg_out")):
        bb = folded_b.get(bn, None)
        if bb is None:
            bb = np.asarray(inputs[bn], f32)
        m[nm] = _b_cols(bb, main_b_scale.get(nm, 1.0))
    return m


def _install_trace_shim():
    """Provide antenv.axon_hooks (NTFF profiling) if the image lacks it."""
    import contextlib
    import ctypes
    import types

    try:
        import antenv.axon_hooks  # noqa: F401
        return
    except ImportError:
        pass
    try:
        import antenv
    except ImportError:
        return
    so_path = "/opt/axon/libaxon_pjrt.so"
    try:
        lib = ctypes.CDLL(so_path)
    except OSError:
        return
    if not hasattr(lib, "axon_start_nrt_profile"):
        return
    lib.axon_start_nrt_profile.argtypes = [ctypes.POINTER(ctypes.c_int64),
                                           ctypes.c_size_t]
    lib.axon_start_nrt_profile.restype = ctypes.c_int64
    lib.axon_stop_nrt_profile.argtypes = [ctypes.c_char_p]
    lib.axon_stop_nrt_profile.restype = ctypes.c_int64

    @contextlib.contextmanager
    def hook(output_dir, device_ids):
        import jax

        jax.devices()
        if device_ids:
            ids = (ctypes.c_int64 * len(device_ids))(*device_ids)
            rc = lib.axon_start_nrt_profile(ids, len(device_ids))
        else:
            rc = lib.axon_start_nrt_profile(None, 0)
        if rc != 0:
            raise RuntimeError(f"axon_start_nrt_profile rc={rc}")
        try:
            yield
        finally:
            n = lib.axon_stop_nrt_profile(str(output_dir).encode())
            print(f"profile: {n} ntff file(s) in {output_dir}",
                  file=sys.stderr)

    mod = types.ModuleType("antenv.axon_hooks")
    mod.get_axon_ntff_profile_hook = lambda: hook
    mod.set_axon_ntff_profile_hook = lambda h: None
    sys.modules["antenv.axon_hooks"] = mod
    antenv.axon_hooks = mod


LAST_RESULTS = None


def kernel(_trace=False, **inputs):
    global _COMPILED, LAST_RESULTS
    from concourse import bass_utils

    if _trace:
        _install_trace_shim()

    if _COMPILED is None:
        _COMPILED = _build()
    nc = _COMPILED

    shared = _prep_shared_inputs(inputs)
    x = np.asarray(inputs["x"], dtype=np.float32)  # [B, T, D]
    in_maps = []
    for c in range(N_CORES):
        g, s = divmod(c, GROUP)
        xT_c = np.ascontiguousarray(x[g, s * S:(s + 1) * S, :].T)
        m = dict(shared)
        m["xT"] = xT_c
        in_maps.append(m)

    LAST_RESULTS = bass_utils.run_bass_kernel_spmd(
        nc, in_maps, core_ids=list(range(N_CORES)), trace=_trace)

    out = np.empty((B, T, D), dtype=np.float32)
    for c in range(N_CORES):
        g, s = divmod(c, GROUP)
        out[g, s * S:(s + 1) * S, :] = LAST_RESULTS.results[c]["outT"].T
    return out
